# revision 40
# baseline (speedup 1.0000x reference)
"""BiMPM forward on 8 Trainium2 NeuronCores (Bass/Tile).

Sharding: 8 cores = (batch b in 0..3) x (side in {p, h}).
  core 2b+0: A = left[b],  B = right[b]   -> mv_p features + agg over mv_p
  core 2b+1: A = right[b], B = left[b]    -> mv_h features + agg over mv_h
Every core runs the same program (SPMD) on its own (A, B) pair:
  ctx BiLSTM over A and B (fw group + bw group, 2 seqs batched per group),
  matching (62 A-side features, feature-major), agg BiLSTM over mv_A
  (final hidden states only). A tiny second launch computes the final FC
  from the gathered per-core agg states.

LSTM recurrence is weight-stationary: per step, 16 (LDWEIGHTS+MATMUL) pairs
produce g.T chunks [128, M] in one PSUM bank; gates evaluated in transposed
layout so h.T feeds the next step's matmul directly (no per-step transpose).
"""
import sys

sys.path.insert(0, '/opt/trn_rl_repo')

import numpy as np
import ml_dtypes

import concourse.bass as bass
import concourse.mybir as mybir
from concourse import tile, masks
from concourse.bass_utils import run_bass_kernel_spmd

F32 = mybir.dt.float32
BF16 = mybir.dt.bfloat16
FP8 = mybir.dt.float8e3  # e3m4
WSCALE = 32.0  # fp8 ctx Whh scale; folded into Wih/bias, undone by act scale
AF = mybir.ActivationFunctionType
OP = mybir.AluOpType
AX = mybir.AxisListType

EPS = 1e-8
B, S, D, H, L, NCLS = 4, 256, 300, 256, 10, 22
GH = 4 * H  # 1024 gates
NCHUNK = 8  # 1024 / 128
AGG_IN = 62
NEG_BIG = -3.0e38

DEBUG_OUTS = False
TRACE = False

# gate chunk order in PSUM columns: i0 i1 f0 f1 o0 o1 g0 g1 (sigmoid 0:6, tanh 6:8)
# host permutes weight/bias gate blocks accordingly (torch i f g o -> i f o g).


class PatchedTC(tile.TileContext):
    """This walrus build rejects instructions carrying more than MAX_WAITS sync
    waits. Tile freely attaches many (one per outstanding producer proc).
    After scheduling, split the excess onto same-engine NOP carriers placed
    immediately before the overloaded instruction."""


MAX_WAITS = 1


def _split_waits(nc, maxw=None):
    if maxw is None:
        maxw = MAX_WAITS
    for f in nc.m.functions:
        for blk in f.blocks:
            insts = blk.instructions  # live list
            out = []
            for inst in insts:
                si = getattr(inst, 'sync_info', None)
                waits = list(si.on_wait) if si is not None else []
                if len(waits) > maxw:
                    excess = waits[:-maxw]
                    for w0 in range(0, len(excess), maxw):
                        nop = _make_nop(nc, inst.engine)
                        nop.sync_info = mybir.SyncInfo(
                            on_wait=excess[w0:w0 + maxw], on_update=[])
                        out.append(nop)
                    inst.sync_info = mybir.SyncInfo(
                        on_wait=waits[-maxw:], on_update=list(si.on_update))
                out.append(inst)
            if len(out) != len(insts):
                insts.clear()
                insts.extend(out)


def _make_nop(nc, engine):
    bi = nc.engines[engine].nop(nofuse=True)
    inst = bi.ins
    cur = nc.cur_bb.bb.instructions
    assert cur and cur[-1].name == inst.name
    cur.pop()
    return inst


# ----------------------------------------------------------------------------
# launch 1 program
# ----------------------------------------------------------------------------

def build_launch1():
    nc = bass.Bass()

    dr = {}
    dr['AT'] = nc.dram_tensor('AT', [D, S], F32, kind='ExternalInput')
    dr['BT'] = nc.dram_tensor('BT', [D, S], F32, kind='ExternalInput')
    for g in range(2):  # 0=fw 1=bw
        dr[f'ctx_WihT_{g}'] = nc.dram_tensor(f'ctx_WihT_{g}', [D, GH], BF16, kind='ExternalInput')
        dr[f'ctx_WhhT_{g}'] = nc.dram_tensor(f'ctx_WhhT_{g}', [H, GH], FP8, kind='ExternalInput')
        dr[f'ctx_b_{g}'] = nc.dram_tensor(f'ctx_b_{g}', [GH], F32, kind='ExternalInput')
        dr[f'agg_WihT_{g}'] = nc.dram_tensor(f'agg_WihT_{g}', [AGG_IN, GH], BF16, kind='ExternalInput')
        dr[f'agg_WhhT_{g}'] = nc.dram_tensor(f'agg_WhhT_{g}', [H, GH], BF16, kind='ExternalInput')
        dr[f'agg_b_{g}'] = nc.dram_tensor(f'agg_b_{g}', [GH], F32, kind='ExternalInput')
    # w*w, padded to 32 cols per perspective: tile a = [w3 w4 w5 w6], tile b = [w7 w8]
    dr['wsqT_a_f32'] = nc.dram_tensor('wsqT_a_f32', [H, 128], F32, kind='ExternalInput')
    dr['wsqT_b_f32'] = nc.dram_tensor('wsqT_b_f32', [H, 64], F32, kind='ExternalInput')
    dr['wsqT_a_bf16'] = nc.dram_tensor('wsqT_a_bf16', [H, 128], BF16, kind='ExternalInput')
    dr['wsqT_b_bf16'] = nc.dram_tensor('wsqT_b_bf16', [H, 64], BF16, kind='ExternalInput')

    encB_dram = [nc.dram_tensor(f'encB_dram_{g}', [S, H], BF16) for g in range(2)]
    # rows staged for partition-broadcast: [2, S]: 0 rnB_cos, 1 rsumA_recip
    brow_dram = [nc.dram_tensor(f'brow_dram_{g}', [2, S], F32) for g in range(2)]
    # maxpool per-perspective B-side recip norms, staged for bulk broadcast
    rlrow_dram = [nc.dram_tensor(f'rlrow_dram_{g}', [L, S], F32) for g in range(2)]

    dr['agg_out'] = nc.dram_tensor('agg_out', [128, 2, 2], F32, kind='ExternalOutput')
    dr['meanA'] = nc.dram_tensor('meanA', [D], F32, kind='ExternalOutput')
    if DEBUG_OUTS:
        dr['mvT_dbg'] = nc.dram_tensor('mvT_dbg', [AGG_IN, S], F32, kind='ExternalOutput')
        dr['encA_dbg'] = nc.dram_tensor('encA_dbg', [2, 128, 2, S + 1], BF16, kind='ExternalOutput')
        dr['encB_dbg'] = nc.dram_tensor('encB_dbg', [2, 128, 2, S + 1], BF16, kind='ExternalOutput')

    with PatchedTC(nc) as tc:
        _emit_core_program(nc, tc, dr, encB_dram, brow_dram, rlrow_dram)
    _split_waits(nc)
    return nc


def _emit_core_program(nc, tc, dr, encB_dram, brow_dram, rlrow_dram):
    with tc.tile_pool(name='persist', bufs=1) as persist:
        # ---------------- identities, weights, inputs ----------------
        id_bf16 = persist.tile([128, 128], BF16, tag='idb', name='idb')
        id_f32 = persist.tile([128, 128], F32, tag='idf', name='idf')
        masks.make_identity(nc, id_bf16[:])
        masks.make_identity(nc, id_f32[:])

        kctx = [(0, 128), (128, 128), (256, 44)]
        # inputs + ctx wih first: the ~30 weight DMAs serialize on one queue
        # at ~650ns each, and preact only needs x + wih.
        xT, xTb = {}, {}
        for nm in ('A', 'B'):
            xT[nm], xTb[nm] = [], []
            for (k0, kn) in kctx:
                t = persist.tile([kn, S], F32, tag=f'x{nm}_{k0}', name=f'x{nm}_{k0}')
                nc.sync.dma_start(t[:], dr[f'{nm}T'][k0:k0 + kn, :])
                xT[nm].append(t)
                tb = persist.tile([kn, S], BF16, tag=f'xb{nm}_{k0}', name=f'xb{nm}_{k0}')
                nc.vector.tensor_copy(tb[:], t[:])
                xTb[nm].append(tb)

        wih, whh, bias = {}, {}, {}
        for g in range(2):
            wih[g] = []
            for (k0, kn) in kctx:
                t = persist.tile([kn, GH], BF16, tag=f'wih{g}_{k0}', name=f'wih{g}_{k0}')
                nc.sync.dma_start(t[:], dr[f'ctx_WihT_{g}'][k0:k0 + kn, :])
                wih[g].append(t)
            t = persist.tile([128, NCHUNK], F32, tag=f'bias{g}', name=f'bias{g}')
            nc.sync.dma_start(t[:], dr[f'ctx_b_{g}'].rearrange('(c p) -> p c', p=128))
            bias[g] = t
        for g in range(2):
            whh[g] = []
            for k in range(2):
                t = persist.tile([128, GH], FP8, tag=f'whh{g}_{k}', name=f'whh{g}_{k}')
                nc.sync.dma_start(t[:], dr[f'ctx_WhhT_{g}'][k * 128:(k + 1) * 128, :])
                whh[g].append(t)

        awih, awhh, abias = {}, {}, {}
        for g in range(2):
            t = persist.tile([AGG_IN, GH], BF16, tag=f'awih{g}', name=f'awih{g}')
            nc.sync.dma_start(t[:], dr[f'agg_WihT_{g}'][:])
            awih[g] = t
            awhh[g] = []
            for k in range(2):
                t = persist.tile([128, GH], BF16, tag=f'awhh{g}_{k}', name=f'awhh{g}_{k}')
                nc.sync.dma_start(t[:], dr[f'agg_WhhT_{g}'][k * 128:(k + 1) * 128, :])
                awhh[g].append(t)
            t = persist.tile([128, NCHUNK], F32, tag=f'abias{g}', name=f'abias{g}')
            nc.sync.dma_start(t[:], dr[f'agg_b_{g}'].rearrange('(c p) -> p c', p=128))
            abias[g] = t

        # wsq_f[ab][k], wsq_b[ab][k]: fp32/bf16 w^2 tiles; ab=0 -> 128 cols, ab=1 -> 64
        wsq_f, wsq_b = {}, {}
        for ab, nch in ((0, 128), (1, 64)):
            wsq_f[ab], wsq_b[ab] = [], []
            abn = 'a' if ab == 0 else 'b'
            for k in range(2):
                t = persist.tile([128, nch], F32, tag=f'wsqf{abn}{k}', name=f'wsqf{abn}{k}')
                nc.sync.dma_start(t[:], dr[f'wsqT_{abn}_f32'][k * 128:(k + 1) * 128, :])
                wsq_f[ab].append(t)
                t = persist.tile([128, nch], BF16, tag=f'wsqb{abn}{k}', name=f'wsqb{abn}{k}')
                nc.sync.dma_start(t[:], dr[f'wsqT_{abn}_bf16'][k * 128:(k + 1) * 128, :])
                wsq_b[ab].append(t)

        ones_col = persist.tile([128, 1], F32, tag='ones', name='ones')
        nc.vector.memset(ones_col[:], 1.0)
        ones_row = persist.tile([1, 128], F32, tag='onesr', name='onesr')
        nc.vector.memset(ones_row[:], 1.0)

        macc = persist.tile([128, 3], F32, tag='macc', name='macc')
        msc = persist.tile([128, 3], F32, tag='msc', name='msc')
        nc.vector.memset(macc[:], 0.0)
        for ki, (k0, kn) in enumerate(kctx):
            nc.vector.tensor_reduce(macc[0:kn, ki:ki + 1], xT['A'][ki][:], axis=AX.X, op=OP.add)
        nc.scalar.activation(msc[:], macc[:], AF.Copy, scale=1.0 / S)
        for ki, (k0, kn) in enumerate(kctx):
            nc.sync.dma_start(dr['meanA'][k0:k0 + kn], msc[0:kn, ki:ki + 1])

        # ---------------- ctx pre-activation ----------------
        # padded [*, WUP:WUP+S, *] live; WUP zero cols either side feed the
        # chunked scan's boundary-chain warmups with exact zero state.
        SP = S + 2 * WUP
        preT = {g: persist.tile([128, NCHUNK, SP, 2], F32, tag=f'pre{g}', name=f'pre{g}') for g in range(2)}
        with tc.tile_pool(name='prepsum', bufs=3, space='PSUM') as pp:
            for g in range(2):
                nc.vector.memset(preT[g][:, :, 0:WUP, :], 0.0)
                nc.vector.memset(preT[g][:, :, S + WUP:SP, :], 0.0)
                for c in range(NCHUNK):
                    ps = pp.tile([128, 2, S], F32, tag='preps', name='preps')
                    n_mm = 0
                    for s, nm in enumerate(('A', 'B')):
                        for ki in range(3):
                            nc.tensor.matmul(
                                ps[:, s, :], wih[g][ki][:, c * 128:(c + 1) * 128], xTb[nm][ki][:],
                                start=(n_mm == 0), stop=(n_mm == 5), skip_group_check=True)
                            n_mm += 1
                    for s in range(2):
                        nc.scalar.activation(
                            preT[g][:, c, WUP:WUP + S, s],
                            ps[:, s, :], AF.Identity, bias=bias[g][:, c:c + 1])

        # ---------------- ctx scans ----------------
        # encT[g]: [128, (half*seq 2M flat, col S+1)] bf16 — flat (hf, m) dim
        # so one strided stt writes all chains x halves x seqs per slot;
        # fw: h_t at col t+1, bw: h_t at col t. enc8: fp8 recurrence copy.
        encT = {g: persist.tile([128, 4, S + 1], BF16, tag=f'enc{g}', name=f'enc{g}') for g in range(2)}
        enc8 = {g: persist.tile([128, 4, S + 1], FP8, tag=f'enc8_{g}', name=f'enc8_{g}') for g in range(2)}

        _emit_scan_chunked(nc, tc, 'ctx', whh, preT, encT, M=2, final_out=None, id_f32=id_f32,
                           enc8=enc8, act_scale=1.0 / WSCALE, KCH=KCH_CTX)

        # ---------------- matching ----------------
        mvT = persist.tile([128, S], F32, tag='mvT', name='mvT')
        _emit_matching(nc, tc, dr, encT, encB_dram, brow_dram, rlrow_dram,
                       wsq_f, wsq_b, ones_col, ones_row, id_bf16, id_f32, mvT)
        mvTb = persist.tile([AGG_IN, S], BF16, tag='mvTb', name='mvTb')
        nc.vector.tensor_copy(mvTb[:], mvT[0:AGG_IN, :])

        if DEBUG_OUTS:
            nc.sync.dma_start(dr['mvT_dbg'][:], mvT[0:AGG_IN, :])
            for g in range(2):
                nc.sync.dma_start(dr['encA_dbg'][g], encT[g][:, 0])
                nc.sync.dma_start(dr['encB_dbg'][g], encT[g][:, 1])

        # ---------------- agg ----------------
        apreT = {g: persist.tile([128, NCHUNK, SP, 1], F32, tag=f'apre{g}', name=f'apre{g}') for g in range(2)}
        with tc.tile_pool(name='aggpp', bufs=3, space='PSUM') as pp:
            for g in range(2):
                nc.vector.memset(apreT[g][:, :, 0:WUP, :], 0.0)
                nc.vector.memset(apreT[g][:, :, S + WUP:SP, :], 0.0)
                for c in range(NCHUNK):
                    ps = pp.tile([128, S], F32, tag='apreps', name='apreps')
                    nc.tensor.matmul(ps[:], awih[g][:, c * 128:(c + 1) * 128], mvTb[:],
                                     start=True, stop=True)
                    # alternate evacuation between Act and DVE (16 copies total)
                    if c % 2 == 0:
                        nc.scalar.activation(apreT[g][:, c, WUP:WUP + S, 0], ps[:], AF.Identity,
                                             bias=abias[g][:, c:c + 1])
                    else:
                        nc.vector.tensor_scalar_add(apreT[g][:, c, WUP:WUP + S, 0], ps[:],
                                                    abias[g][:, c:c + 1])

        aencT = {g: persist.tile([128, 2, S + 1], BF16, tag=f'aenc{g}', name=f'aenc{g}') for g in range(2)}

        final_h = persist.tile([128, 2, 2], F32, tag='finalh', name='finalh')  # (group, half)
        _emit_scan_chunked(nc, tc, 'agg', awhh, apreT, aencT, M=1, final_out=final_h, id_f32=id_f32, KCH=KCH_AGG)
        nc.sync.dma_start(dr['agg_out'][:], final_h[:])


KCH = 16  # chunks per direction (must divide S)
KCH_CTX = 16
KCH_AGG = 32  # finer chunks for the agg scan (ops are half-size at M=1)
WUP = 16  # warmup steps per chunk; chunk-boundary state error decays ~0.67^WUP


def _emit_scan_chunked(nc, tc, name, whh, preT, encT, M, final_out, id_f32,
                       enc8=None, act_scale=1.0, KCH=KCH):
    """Lockstep chunked scan. Per direction, KCH independent chains (one per
    seq chunk) advance together, so each per-step engine op covers all KCH
    chains in one strided-AP instruction and the ~1.8us/step cross-engine
    dependence chain is amortized KCH-fold. Each chain runs WUP zero-state
    warmup steps on the real pre-activations preceding its chunk; chain 0
    fw (and the mirrored bw chain) warms up on preT's zero padding, which
    reproduces the exact zero initial state (zero pre -> c,h stay 0).

    preT[g]: [128, NCHUNK, S + 2*WUP, M], cols [0,WUP) and [S+WUP,S+2WUP)
    zero. encT/enc8 layouts unchanged; fw chain j owns real cols
    jC+1..jC+C (bw mirrored). Warmup h lives in a scratch block
    [128, M, 2, KCH, WUP+1] (col 0 = zero init), fp8 for ctx, bf16 agg.
    Gate math is the same tanh-trick as _emit_scan, KCH-wide."""
    C = S // KCH
    SLOTS = C + WUP
    with (
        tc.tile_pool(name=f'{name}_ps0', bufs=2, space='PSUM') as pp0,
        tc.tile_pool(name=f'{name}_ps1', bufs=2, space='PSUM') as pp1,
        tc.tile_pool(name=f'{name}_sb', bufs=3) as sb,
        tc.tile_pool(name=f'{name}_scr', bufs=1) as scrp,
    ):
        pps = {0: pp0, 1: pp1}
        rec_dt = FP8 if enc8 is not None else BF16
        rec = {g: (enc8[g] if enc8 is not None else encT[g]) for g in range(2)}
        # stt/activation outputs must be <=3D (birverifier): gate tiles are
        # [128, KCH, gate-cols] and enc writes are split per h-half.
        scr, c_state = {}, {}
        for g in range(2):
            t = scrp.tile([128, WUP + 1, KCH, 2 * M], rec_dt, tag=f'scr{g}', name=f'scr{g}')
            nc.vector.memset(t[:, 0, :, :], 0.0)
            scr[g] = t
            c_state[g] = scrp.tile([128, KCH, 2 * M], F32, tag=f'c{g}', name=f'c{g}')
            nc.vector.memset(c_state[g][:], 0.0)
        for s in range(SLOTS):
            for g in range(2):
                if g == 0:
                    pre_off = s
                    wr_off = s - WUP + 1
                    rd_off = s - WUP
                else:
                    pre_off = C - 1 + 2 * WUP - s
                    wr_off = C - 1 + WUP - s
                    rd_off = C + WUP - s
                nsl = (KCH - 1) * C + 1  # strided-slice span over chains
                ps = pps[g].tile([128, KCH, NCHUNK * M], F32, tag=f'gsum{g}', name=f'gsum{g}')
                for c in range(NCHUNK):
                    nc.tensor.matmul(
                        ps[:, :, c * M:(c + 1) * M], id_f32[:],
                        preT[g][:, c, pre_off:pre_off + nsl:C, :],
                        start=(c == 0), stop=False, skip_group_check=True)
                n_mm = 0
                for c in range(NCHUNK):
                    for k in range(2):
                        if s <= WUP:
                            rsrc = scr[g][:, s, :, k * M:(k + 1) * M]
                        else:
                            rsrc = rec[g][:, k * M:(k + 1) * M,
                                          rd_off:rd_off + nsl:C].transpose([0, 2, 1])
                        nc.tensor.matmul(
                            ps[:, :, c * M:(c + 1) * M],
                            whh[g][k][:, c * 128:(c + 1) * 128],
                            rsrc, start=False, stop=(n_mm == 15), skip_group_check=True)
                        n_mm += 1
                T = sb.tile([128, KCH, NCHUNK * M], F32, tag=f'th8{g}', name=f'th8{g}')
                nc.scalar.activation(T[:], ps[:], AF.Tanh, scale=act_scale)
                cs = c_state[g]
                Bt = sb.tile([128, KCH, 2 * M], F32, tag=f'B{g}', name=f'B{g}')
                nc.vector.scalar_tensor_tensor(
                    Bt[:], T[:, :, 0:2 * M], 1.0, T[:, :, 6 * M:8 * M],
                    op0=OP.add, op1=OP.mult)
                At = sb.tile([128, KCH, 2 * M], F32, tag=f'A{g}', name=f'A{g}')
                nc.vector.scalar_tensor_tensor(
                    At[:], T[:, :, 2 * M:4 * M], 1.0, cs[:], op0=OP.add, op1=OP.mult)
                nc.vector.scalar_tensor_tensor(
                    cs[:], At[:], 0.5, Bt[:], op0=OP.mult, op1=OP.add)
                TH = sb.tile([128, KCH, 2 * M], F32, tag=f'TH{g}', name=f'TH{g}')
                nc.scalar.activation(TH[:], cs[:], AF.Tanh, scale=0.5)
                if s < WUP:
                    # warmup: only the recurrence copy (fp8 ctx / bf16 agg)
                    nc.vector.scalar_tensor_tensor(
                        scr[g][:, s + 1, :, :], T[:, :, 4 * M:6 * M], 1.0, TH[:],
                        op0=OP.add, op1=OP.mult)
                else:
                    wsl = slice(wr_off, wr_off + nsl, C)
                    nc.vector.scalar_tensor_tensor(
                        rec[g][:, :, wsl].transpose([0, 2, 1]),
                        T[:, :, 4 * M:6 * M], 1.0, TH[:], op0=OP.add, op1=OP.mult)
                    if enc8 is not None:
                        nc.vector.scalar_tensor_tensor(
                            encT[g][:, :, wsl].transpose([0, 2, 1]),
                            T[:, :, 4 * M:6 * M], 1.0, TH[:], op0=OP.add, op1=OP.mult)
                if final_out is not None and s == SLOTS - 1:
                    ec = KCH - 1 if g == 0 else 0
                    nc.vector.scalar_tensor_tensor(
                        final_out[:, g, :], T[:, ec, 4 * M:6 * M], 1.0, TH[:, ec, :],
                        op0=OP.add, op1=OP.mult)


def _emit_scan(nc, tc, name, whh, preT, encT, M, final_out, id_f32,
               enc8=None, act_scale=1.0):
    """Interleaved fw/bw scan groups, tanh-only gate math.

    Host prep scales gate rows so that: i,f,o pre-acts arrive as x/2 and the
    whh matmul consumes the 2h encoding. Per step:
      PSUM prefilled with pre via identity matmuls, whh matmuls accumulate,
      T = tanh(psum)   (sig(x) = (T+1)/2 for i,f,o cols; tg for g cols)
      state W = 2c:  W' = 0.5*(t_f+1)*W + (t_i+1)*tg
      TH = tanh(0.5*W') = tanh(c')
      enc h2x = (t_o+1)*TH = 2h
    encT[g]: [128, (M seq, 2 half, S+1)] bf16 storing 2h. preT[g]: [128, (8, S, M)] f32.
    whh[g]: 2 k-tiles [128, 1024] bf16, gate chunks i0i1 f0f1 o0o1 g0g1,
    rows pre-scaled by 0.25 (i,f,o) / 0.5 (g)."""
    with (
        tc.tile_pool(name=f'{name}_ps0', bufs=2, space='PSUM') as pp0,
        tc.tile_pool(name=f'{name}_ps1', bufs=2, space='PSUM') as pp1,
        tc.tile_pool(name=f'{name}_sb', bufs=3) as sb,
    ):
        pps = {0: pp0, 1: pp1}
        c_state = {}
        for g in range(2):
            c_state[g] = sb.tile([128, 2 * M], F32, tag=f'c{g}', name=f'c{g}')  # (half, m) order
            nc.vector.memset(c_state[g][:], 0.0)
        for step in range(S):
            for g in range(2):
                t = step if g == 0 else S - 1 - step
                rd = t if g == 0 else t + 1
                wr = t + 1 if g == 0 else t
                ps = pps[g].tile([128, NCHUNK * M], F32, tag=f'gsum{g}', name=f'gsum{g}')
                psv = ps[:].rearrange('p (c m) -> p c m', c=NCHUNK)
                # prefill psum with pre (identity matmuls share one LDW)
                for c in range(NCHUNK):
                    nc.tensor.matmul(psv[:, c, :], id_f32[:], preT[g][:, c, t, :],
                                     start=(c == 0), stop=False, skip_group_check=True)
                rsrc = enc8[g] if enc8 is not None else encT[g]
                n_mm = 0
                for c in range(NCHUNK):
                    for k in range(2):
                        nc.tensor.matmul(
                            ps[:, c * M:(c + 1) * M],
                            whh[g][k][:, c * 128:(c + 1) * 128],
                            rsrc[:, :, k, rd],
                            start=False, stop=(n_mm == 15), skip_group_check=True)
                        n_mm += 1
                T = sb.tile([128, NCHUNK * M], F32, tag=f'th8{g}', name=f'th8{g}')
                nc.scalar.activation(T[:], ps[:], AF.Tanh, scale=act_scale)
                cs = c_state[g]
                A = sb.tile([128, 2 * M], F32, tag=f'A{g}', name=f'A{g}')
                nc.vector.scalar_tensor_tensor(
                    A[:], T[:, 2 * M:4 * M], 1.0, cs[:], op0=OP.add, op1=OP.mult)
                B = sb.tile([128, 2 * M], F32, tag=f'B{g}', name=f'B{g}')
                nc.vector.scalar_tensor_tensor(
                    B[:], T[:, 0:2 * M], 1.0, T[:, 6 * M:8 * M], op0=OP.add, op1=OP.mult)
                nc.vector.scalar_tensor_tensor(
                    cs[:], A[:], 0.5, B[:], op0=OP.mult, op1=OP.add)
                TH = sb.tile([128, 2 * M], F32, tag=f'TH{g}', name=f'TH{g}')
                nc.scalar.activation(TH[:], cs[:], AF.Tanh, scale=0.5)
                # h2x = (t_o+1)*TH; encT dest dims (m, h) permuted to (h, m)
                if enc8 is not None:
                    # fp8 copy feeds the next step's matmul — emit first (on
                    # the critical chain); bf16 copy (for matching) follows.
                    nc.vector.scalar_tensor_tensor(
                        enc8[g][:, :, :, wr].transpose([0, 2, 1]),
                        T[:, 4 * M:6 * M].rearrange('p (h m) -> p h m', h=2),
                        1.0, TH[:].rearrange('p (h m) -> p h m', h=2),
                        op0=OP.add, op1=OP.mult)
                hout = encT[g][:, :, :, wr].transpose([0, 2, 1])
                nc.vector.scalar_tensor_tensor(
                    hout, T[:, 4 * M:6 * M].rearrange('p (h m) -> p h m', h=2),
                    1.0, TH[:].rearrange('p (h m) -> p h m', h=2),
                    op0=OP.add, op1=OP.mult)
                if final_out is not None and step == S - 1:
                    # 2h; host scales agg_out by 0.5
                    nc.vector.scalar_tensor_tensor(
                        final_out[:, g, :], T[:, 4 * M:6 * M], 1.0, TH[:],
                        op0=OP.add, op1=OP.mult)


def _emit_matching(nc, tc, dr, encT, encB_dram, brow_dram, rlrow_dram,
                   wsq_f, wsq_b, ones_col, ones_row, id_bf16, id_f32, mvT):
    """A-side matching features into mvT rows 0:62 ([feat, S] f32).

    rows: 0 cos-max, 1 cos-mean, 2:12 maxpool-f(w3), 12:22 maxpool-b(w4),
          22:32 attentive-f(w5), 32:42 attentive-b(w6),
          42:52 max-attentive-f(w7), 52:62 max-attentive-b(w8)
    w-set s in 0..5 lives in wsq tile s//4 at col offset 32*(s%4), 10 cols wide.

    Emitted as phase A (norms, att, transposes, attentive-mean) for both
    directions, then phase B (maxpool, max-attentive, final cosines) for
    both: B's engine queues then never drain while the other direction's
    A-phase DMA chains are in flight.
    """
    with tc.tile_pool(name='m_sb', bufs=1) as msb:
        colfeat = msb.tile([128, 2, 22], F32, tag='colfeat', name='colfeat')
        att = {}
        rl_all = {}
        nsA = {}
        rnAs0 = {}
        meanT = {}

        for g in range(2):
            c0 = 1 if g == 0 else 0
            eAT = [encT[g][:, 2 * k, c0:c0 + S] for k in range(2)]  # [128, S] bf16 views
            eBT = [encT[g][:, 2 * k + 1, c0:c0 + S] for k in range(2)]

            # --- squares (scalar engine: DVE is the matching bottleneck)
            sqA = [msb.tile([128, S], F32, tag=f'sqA{k}', name=f'sqA{k}') for k in range(2)]
            sqB = [msb.tile([128, S], F32, tag=f'sqB{k}', name=f'sqB{k}') for k in range(2)]
            for k in range(2):
                nc.scalar.activation(sqA[k][:], eAT[k], AF.Square)
                nc.scalar.activation(sqB[k][:], eBT[k], AF.Square)

            # --- norm sets: nsqA/B for all 6 w-sets (padded 32 rows each, 2 tiles)
            # recip norms only for set tile 0 (maxpool); sets 2..5 use nsA raw.
            nsA[g] = [msb.tile([128, S], F32, tag=f'nsA{g}{ab}', name=f'nsA{g}{ab}') for ab in range(2)]
            rnAs0[g] = msb.tile([128, S], F32, tag=f'rnAs0{g}', name=f'rnAs0{g}')
            rnBs0 = msb.tile([128, S], F32, tag='rnBs0', name='rnBs0')
            with tc.tile_pool(name=f'mns{g}', bufs=2, space='PSUM') as mps:
                for ab, nch in ((0, 128), (1, 64)):
                    ps = mps.tile([128, S], F32, tag='nset', name='nset')
                    for k in range(2):
                        nc.tensor.matmul(ps[0:nch, :], wsq_f[ab][k][:], sqA[k][:],
                                         start=(k == 0), stop=(k == 1), skip_group_check=True)
                    nc.scalar.copy(nsA[g][ab][0:nch, :], ps[0:nch, :])
                    if ab == 0:
                        nc.scalar.activation(rnAs0[g][:], ps[:], AF.Sqrt)
                        nc.vector.tensor_scalar_max(rnAs0[g][:], rnAs0[g][:], EPS)
                        nc.vector.reciprocal(rnAs0[g][:], rnAs0[g][:])
                        ps2 = mps.tile([128, S], F32, tag='nset', name='nset')
                        for k in range(2):
                            nc.tensor.matmul(ps2[:], wsq_f[0][k][:], sqB[k][:],
                                             start=(k == 0), stop=(k == 1), skip_group_check=True)
                        nc.scalar.activation(rnBs0[:], ps2[:], AF.Sqrt)
                        nc.vector.tensor_scalar_max(rnBs0[:], rnBs0[:], EPS)
                        nc.vector.reciprocal(rnBs0[:], rnBs0[:])

            # --- stage + bulk-broadcast the maxpool B-side recip norms (set g):
            # one [L, S] DRAM write + two 5-row broadcasts (a 10-row f32 one
            # exceeds the ISA length limit), hoisted out of the maxpool loop.
            base = 32 * g
            rl_all[g] = msb.tile([128, L, S], F32, tag=f'rlall{g}', name=f'rlall{g}')
            nc.sync.dma_start(rlrow_dram[g][:], rnBs0[base:base + L, :])
            for hl in range(2):
                nc.sync.dma_start(
                    rl_all[g][:, hl * 5:(hl + 1) * 5, :],
                    rlrow_dram[g][hl * 5:(hl + 1) * 5, :].partition_broadcast(128))

            # --- cos recip norms: rnA [p,1] per-partition; rnB as a [1, S]
            # row (PE column-sum) then PE-broadcast to [128, S] PSUM — no
            # DRAM round trip (the old brow chain cost ~4us of DMA latency)
            rnA = msb.tile([128, 2], F32, tag='rnA', name='rnA')
            rnB_row = msb.tile([1, S], F32, tag='rnBrow', name='rnBrow')
            with tc.tile_pool(name=f'mn2{g}', bufs=2, space='PSUM') as mps:
                ps = mps.tile([128, 2], F32, tag='nsq', name='nsq')
                for pt in range(2):
                    for k in range(2):
                        nc.tensor.matmul(ps[:, pt:pt + 1],
                                         sqA[k][:, pt * 128:(pt + 1) * 128],
                                         ones_col[:], start=(k == 0), stop=(k == 1),
                                         skip_group_check=True)
                sq_ = msb.tile([128, 2], F32, tag='nsq_s', name='nsq_s')
                nc.scalar.activation(sq_[:], ps[:], AF.Sqrt)
                nc.vector.tensor_scalar_max(sq_[:], sq_[:], EPS)
                nc.vector.reciprocal(rnA[:], sq_[:])
                psr = mps.tile([1, S], F32, tag='nsqr', name='nsqr')
                for k in range(2):
                    nc.tensor.matmul(psr[:], ones_col[:], sqB[k][:],
                                     start=(k == 0), stop=(k == 1), skip_group_check=True)
                nc.scalar.activation(rnB_row[:], psr[:], AF.Sqrt)
                nc.vector.tensor_scalar_max(rnB_row[:], rnB_row[:], EPS)
                nc.vector.reciprocal(rnB_row[:], rnB_row[:])

            # --- att = num * rnA[p] * rnB[q]
            att[g] = [msb.tile([128, S], F32, tag=f'att{g}{pt}', name=f'att{g}{pt}')
                      for pt in range(2)]
            rsum = msb.tile([128, 2], F32, tag='rsum', name='rsum')
            with (
                tc.tile_pool(name=f'ma{g}', bufs=2, space='PSUM') as mps,
                tc.tile_pool(name=f'mab{g}', bufs=1, space='PSUM') as bcp,
            ):
                rnB_ps = bcp.tile([128, S], F32, tag='rnBps', name='rnBps')
                nc.tensor.matmul(rnB_ps[:], ones_row[:], rnB_row[:],
                                 start=True, stop=True, skip_group_check=True)
                # stt may read only one PSUM operand; evacuate the broadcast
                rnB_bc = msb.tile([128, S], F32, tag='rnBbc', name='rnBbc')
                nc.scalar.copy(rnB_bc[:], rnB_ps[:])
                for pt in range(2):
                    nps = mps.tile([128, S], F32, tag='num', name='num')
                    for k in range(2):
                        nc.tensor.matmul(nps[:], eAT[k][:, pt * 128:(pt + 1) * 128],
                                         eBT[k], start=(k == 0), stop=(k == 1),
                                         skip_group_check=True)
                    nc.vector.scalar_tensor_tensor(
                        att[g][pt][:], nps[:], rnA[:, pt:pt + 1], rnB_bc[:],
                        op0=OP.mult, op1=OP.mult)
                for pt in range(2):
                    if g == 0:  # cos max/mean features use att_fw only
                        nc.vector.tensor_reduce(colfeat[:, pt, 0:1], att[g][pt][:],
                                                axis=AX.X, op=OP.max)
                    nc.vector.tensor_reduce(rsum[:, pt:pt + 1], att[g][pt][:],
                                            axis=AX.X, op=OP.add)
                if g == 0:
                    nc.scalar.activation(colfeat[:, :, 1], rsum[:], AF.Copy, scale=1.0 / S)
                # NOTE: the attentive-mean 1/max(att.sum, EPS) factor is a
                # positive per-row scalar that cancels in the downstream
                # cosine (mp_match) — no need to compute or apply it.

            # --- transposes: enc_B [q, h] + attT [q, p]; both halves land in
            # one PSUM tile so each needs a single evacuation copy
            encB = [msb.tile([128, S], BF16, tag=f'encB{qt}', name=f'encB{qt}') for qt in range(2)]
            attT = [msb.tile([128, S], BF16, tag=f'attT{qt}', name=f'attT{qt}') for qt in range(2)]
            with tc.tile_pool(name=f'mt{g}', bufs=2, space='PSUM') as mps:
                for qt in range(2):
                    tpb = mps.tile([128, 2, 128], BF16, tag='tpb', name='tpb')
                    for hf in range(2):
                        nc.tensor.transpose(tpb[:, hf, :], eBT[hf][:, qt * 128:(qt + 1) * 128],
                                            id_bf16[:])
                    nc.scalar.copy(encB[qt][:], tpb[:].rearrange('p h n -> p (h n)'))
                    tpf = mps.tile([128, 2, 128], F32, tag='tpf', name='tpf')
                    for pt in range(2):
                        nc.tensor.transpose(tpf[:, pt, :], att[g][pt][:, qt * 128:(qt + 1) * 128],
                                            id_f32[:])
                    nc.scalar.copy(attT[qt][:], tpf[:].rearrange('p h n -> p (h n)'))
                    nc.sync.dma_start(encB_dram[g][qt * 128:(qt + 1) * 128, :], encB[qt][:])

            # --- attentive mean (transposed, unnormalized): meanT[h, p]
            meanT[g] = [msb.tile([128, S], BF16, tag=f'meanT{g}{ht}', name=f'meanT{g}{ht}')
                        for ht in range(2)]
            with tc.tile_pool(name=f'mm{g}', bufs=2, space='PSUM') as mps:
                for ht in range(2):
                    mp = mps.tile([128, S], F32, tag='meanps', name='meanps')
                    for qt in range(2):
                        nc.tensor.matmul(mp[:], encB[qt][:, ht * 128:(ht + 1) * 128], attT[qt][:],
                                         start=(qt == 0), stop=(qt == 1), skip_group_check=True)
                    nc.scalar.copy(meanT[g][ht][:], mp[:])

        # ---- phase B: one fused instruction stream per direction.
        # The max-attentive units keep Act/Pool/DVE all saturated; the
        # DVE-only maxpool iterations and the previous direction's final
        # cosine sets are sprinkled between vb blocks so the engines never
        # drain at section boundaries.
        QB = 16
        pend = []  # deferred emitters from the previous direction

        def _emit_xt_transpose(g, xacc_g, xT_g):
            with tc.tile_pool(name=f'mxt{g}', bufs=2, space='PSUM') as mps:
                for ht in range(2):
                    for pt in range(2):
                        tp = mps.tile([128, 128], BF16, tag='tpx', name='tpx')
                        nc.tensor.transpose(tp[:], xacc_g[pt][:, ht * 128:(ht + 1) * 128],
                                            id_bf16[:])
                        nc.scalar.copy(xT_g[ht][:, pt * 128:(pt + 1) * 128], tp[:])

        def _emit_final_set(g, eAT_g, vT, set_, row0):
            ab, off = divmod(set_, 4)
            off *= 32
            prod = [msb.tile([128, S], BF16, tag=f'prod{k}', name=f'prod{k}') for k in range(2)]
            vsq = [msb.tile([128, S], F32, tag=f'vsq{k}', name=f'vsq{k}') for k in range(2)]
            for k in range(2):
                nc.vector.tensor_tensor(prod[k][:], eAT_g[k], vT[k][:], OP.mult)
                nc.scalar.activation(vsq[k][:], vT[k][:], AF.Square)
            # stage this set's A-norms at base partition 0 (engine ops need equal bases)
            n1s = msb.tile([L, S], F32, tag='n1s', name='n1s')
            nc.sync.dma_start(n1s[:], nsA[g][ab][off:off + L, :])
            feat = msb.tile([L, S], F32, tag='feat', name='feat')
            with tc.tile_pool(name=f'mf{g}{row0}', bufs=1, space='PSUM') as mps:
                nump = mps.tile([128, S], F32, tag='nump', name='nump')
                n2p = mps.tile([128, S], F32, tag='n2p', name='n2p')
                for k in range(2):
                    nc.tensor.matmul(nump[0:L, :], wsq_b[ab][k][:, off:off + L],
                                     prod[k][:], start=(k == 0), stop=(k == 1),
                                     skip_group_check=True)
                    nc.tensor.matmul(n2p[0:L, :], wsq_f[ab][k][:, off:off + L],
                                     vsq[k][:], start=(k == 0), stop=(k == 1),
                                     skip_group_check=True)
                den = msb.tile([128, S], F32, tag='den', name='den')
                nc.vector.tensor_tensor(den[0:L, :], n2p[0:L, :], n1s[:], OP.mult)
                nc.scalar.activation(den[0:L, :], den[0:L, :], AF.Sqrt)
                nc.vector.tensor_scalar_max(den[0:L, :], den[0:L, :], EPS)
                nc.vector.reciprocal(den[0:L, :], den[0:L, :])
                nc.vector.tensor_tensor(feat[:], nump[0:L, :], den[0:L, :], OP.mult)
            # place rows via DMA (arbitrary partition offset)
            nc.sync.dma_start(mvT[row0:row0 + L, :], feat[:])

        for g in range(2):
            c0 = 1 if g == 0 else 0
            eAT = [encT[g][:, 2 * k, c0:c0 + S] for k in range(2)]
            eBT = [encT[g][:, 2 * k + 1, c0:c0 + S] for k in range(2)]
            base = 32 * g

            rnA_l = msb.tile([128, 2, L], F32, tag='rnAl', name='rnAl')
            mp_acc = msb.tile([128, 2, L], F32, tag='mpacc', name='mpacc')
            scr = msb.tile([128, S], F32, tag='mpscr', name='mpscr')
            xacc = [msb.tile([128, S], BF16, tag=f'xacc{g}{pt}', name=f'xacc{g}{pt}')
                    for pt in range(2)]
            acc2 = [msb.tile([128, 2, S], BF16, tag=f'acc2_{pt}', name=f'acc2_{pt}')
                    for pt in range(2)]
            for pt in range(2):
                nc.vector.memset(acc2[pt][:], NEG_BIG)
            # q-pairs share one [128, 2, H] tmp tile (Act/Pool/DVE each fill a
            # slot) and ONE DVE TT-max merges both units at 327ns/pair; per 16
            # pairs: 13 (A,P) + 2 (A,D) + 1 (D,D) -> Act 15x398, Pool 13x450,
            # DVE 16x327+4x127 -- all three within 4%.
            PAIRS = ['AP'] * 13 + ['AD'] * 2 + ['DD']  # 16 q-pairs / 32 units
            with (
                tc.tile_pool(name=f'mp{g}', bufs=3, space='PSUM') as mps,
                tc.tile_pool(name=f'mpb{g}', bufs=2) as bcp,
                tc.tile_pool(name=f'mx{g}', bufs=2) as vbp,
                tc.tile_pool(name=f'mxt{g}t', bufs=12) as tmpp,
            ):
                for pt in range(2):
                    tpf = mps.tile([128, L], F32, tag='tprn', name='tprn')
                    nc.tensor.transpose(tpf[:], rnAs0[g][base:base + L, pt * 128:(pt + 1) * 128],
                                        id_f32[base:base + L, base:base + L])
                    nc.scalar.copy(rnA_l[:, pt, :], tpf[:])
                for bi, q0 in enumerate(range(0, S, QB)):
                    vb = vbp.tile([128, QB, H], BF16, tag='vbc', name='vbc')
                    nc.sync.dma_start(vb[:], encB_dram[g][q0:q0 + QB, :].partition_broadcast(128))
                    for qp in range(QB // 2):
                        qq0, qq1 = 2 * qp, 2 * qp + 1
                        for pt in range(2):
                            kind = PAIRS[(2 * qp + pt) % 16]
                            tmp2 = tmpp.tile([128, 2, H], BF16, tag='xtmp', name='xtmp')
                            for sl, (qq, r) in enumerate(((qq0, kind[0]), (qq1, kind[1]))):
                                q = q0 + qq
                                if r == 'A':
                                    nc.scalar.activation(tmp2[:, sl, :], vb[:, qq, :], AF.Copy,
                                                         scale=att[g][pt][:, q:q + 1])
                                elif r == 'P':
                                    nc.gpsimd.tensor_scalar(tmp2[:, sl, :], vb[:, qq, :],
                                                            att[g][pt][:, q:q + 1], None,
                                                            op0=OP.mult)
                                else:
                                    nc.vector.tensor_scalar(tmp2[:, sl, :], vb[:, qq, :],
                                                            att[g][pt][:, q:q + 1], None,
                                                            op0=OP.mult)
                            nc.vector.tensor_tensor(acc2[pt][:], tmp2[:], acc2[pt][:], OP.max)
                    if 1 <= bi <= L:  # one maxpool perspective per vb block
                        l = bi - 1
                        wa = [bcp.tile([128, S], BF16, tag=f'wa{k}', name=f'wa{k}')
                              for k in range(2)]
                        for k in range(2):
                            nc.vector.tensor_scalar_mul(
                                wa[k][:], eAT[k], wsq_f[0][k][:, base + l:base + l + 1])
                        for pt in range(2):
                            nps = mps.tile([128, S], F32, tag='mpnum', name='mpnum')
                            for k in range(2):
                                nc.tensor.matmul(nps[:], wa[k][:, pt * 128:(pt + 1) * 128],
                                                 eBT[k], start=(k == 0), stop=(k == 1),
                                                 skip_group_check=True)
                            # (tensor_tensor_reduce would fuse these, but this
                            # walrus build rejects it: "ISA wrong length")
                            nc.vector.tensor_tensor(scr[:], nps[:], rl_all[g][:, l, :], OP.mult)
                            nc.vector.tensor_reduce(mp_acc[:, pt, l:l + 1], scr[:],
                                                    axis=AX.X, op=OP.max)
                    elif bi == L + 1:
                        for pt in range(2):
                            nc.vector.tensor_tensor(colfeat[:, pt, 2 + g * L:2 + (g + 1) * L],
                                                    mp_acc[:, pt, :], rnA_l[:, pt, :], OP.mult)
                    elif pend:
                        pend.pop(0)()
                for pt in range(2):
                    nc.vector.tensor_tensor(xacc[pt][:], acc2[pt][:, 0, :],
                                            acc2[pt][:, 1, :], OP.max)
            while pend:
                pend.pop(0)()

            # defer this direction's xacc transposes + final cosine sets into
            # the next stream (or the tail, for the last direction)
            xT_ = [msb.tile([128, S], BF16, tag=f'xT{ht}', name=f'xT{ht}') for ht in range(2)]
            pend = [
                (lambda g=g, xacc=xacc, xT_=xT_: _emit_xt_transpose(g, xacc, xT_)),
                (lambda g=g, eAT=eAT: _emit_final_set(g, eAT, meanT[g], 2 + g, 22 + g * L)),
                (lambda g=g, eAT=eAT, xT_=xT_: _emit_final_set(g, eAT, xT_, 4 + g, 42 + g * L)),
            ]
        while pend:
            pend.pop(0)()

        # --- transpose column features into mvT rows 0:22
        with tc.tile_pool(name='cf_ps', bufs=2, space='PSUM') as cps:
            for pt in range(2):
                tp = cps.tile([22, 128], F32, tag='tpcf', name='tpcf')
                nc.tensor.transpose(tp[:], colfeat[:, pt, :], id_f32[:])
                nc.scalar.copy(mvT[0:22, pt * 128:(pt + 1) * 128], tp[:])

NT2 = 13  # launch2 k-tiles; host pads NX 1626 -> 13*128
NH2 = 64  # hidden chunk per core: 512 / 8


def build_launch2():
    """Final FC, sharded over the 8 cores: core i computes its 64-wide slice
    of h = tanh(W1 x + b1) and the partial y = W2[:, slice] @ h_slice; the
    host sums the 8 partials (+b2). y is linear in h, so partials add."""
    nc = bass.Bass()
    NXP = NT2 * 128  # 1664 (zero-padded)
    xT = nc.dram_tensor('xT', [NXP, B], BF16, kind='ExternalInput')
    w1T = nc.dram_tensor('w1T', [NXP, NH2], BF16, kind='ExternalInput')
    b1 = nc.dram_tensor('b1', [NH2], F32, kind='ExternalInput')
    w2T = nc.dram_tensor('w2T', [NH2, NCLS], F32, kind='ExternalInput')
    yT = nc.dram_tensor('yT', [NCLS, B], F32, kind='ExternalOutput')

    with PatchedTC(nc) as tc:
        with (
            tc.tile_pool(name='sb', bufs=1) as sb,
            tc.tile_pool(name='ps', bufs=2, space='PSUM') as pp,
        ):
            xts = sb.tile([128, NT2, B], BF16, tag='x', name='x')
            nc.sync.dma_start(xts[:], xT.rearrange('(t p) n -> p t n', p=128))
            w1s = sb.tile([128, NT2, NH2], BF16, tag='w1', name='w1')
            nc.sync.dma_start(w1s[:], w1T.rearrange('(t p) n -> p t n', p=128))
            b1t = sb.tile([NH2, 1], F32, tag='b1', name='b1')
            nc.sync.dma_start(b1t[:], b1.rearrange('(n o) -> n o', o=1))
            w2s = sb.tile([NH2, NCLS], F32, tag='w2', name='w2')
            nc.sync.dma_start(w2s[:], w2T[:, :])
            hT = sb.tile([NH2, B], F32, tag='hT', name='hT')
            ps = pp.tile([NH2, B], F32, tag='h', name='h')
            for i in range(NT2):
                nc.tensor.matmul(ps[:], w1s[:, i, :], xts[:, i, :],
                                 start=(i == 0), stop=(i == NT2 - 1),
                                 skip_group_check=True)
            nc.scalar.activation(hT[:], ps[:], AF.Tanh, bias=b1t[:])
            ps2 = pp.tile([NCLS, B], F32, tag='y', name='y')
            nc.tensor.matmul(ps2[:], w2s[:], hT[:], start=True, stop=True,
                             skip_group_check=True)
            yt = sb.tile([NCLS, B], F32, tag='yt', name='yt')
            nc.scalar.copy(yt[:], ps2[:])
            nc.sync.dma_start(yT[:], yt[:])
    _split_waits(nc)
    return nc


# ----------------------------------------------------------------------------
# host orchestration
# ----------------------------------------------------------------------------

_cache = {}


def _gate_perm():
    # torch gate order (i, f, g, o) blocks of H -> chip order (i, f, o, g),
    # and within each gate the two 128-halves stay in order.
    idx = np.arange(GH).reshape(4, H)
    return np.concatenate([idx[0], idx[1], idx[3], idx[2]])


def _prep_host(inputs):
    bf = ml_dtypes.bfloat16
    perm = _gate_perm()
    # tanh-trick row scaling in chip gate order (i, f, o, g blocks of H):
    # Wih/bias rows: x/2 for sigmoid gates (i,f,o); Whh additionally consumes
    # the 2h encoding, so its rows get an extra 0.5.
    sig_rows = np.concatenate([np.full(3 * H, 0.5, np.float32),
                               np.ones(H, np.float32)])
    pr = {}
    for g, d in ((0, 'f'), (1, 'b')):
        for pref in ('ctx', 'agg'):
            wih = np.asarray(inputs[f'{pref}_Wih_{d}'], np.float32)[perm]  # [1024, IN]
            whh = np.asarray(inputs[f'{pref}_Whh_{d}'], np.float32)[perm]
            bb = np.asarray(inputs[f'{pref}_b_{d}'], np.float32)[perm]
            wih = wih * sig_rows[:, None]
            whh = whh * (0.5 * sig_rows)[:, None]
            bb = bb * sig_rows
            if pref == 'ctx':
                # fp8 recurrence: whh scaled up by WSCALE (kept in fp8 normal
                # range); wih/bias scaled to match, undone by the act scale.
                wih = wih * WSCALE
                bb = bb * WSCALE
                pr[f'{pref}_WhhT_{g}'] = np.ascontiguousarray(
                    (whh * WSCALE).T).astype(ml_dtypes.float8_e3m4)
            else:
                pr[f'{pref}_WhhT_{g}'] = np.ascontiguousarray(whh.T).astype(bf)
            pr[f'{pref}_WihT_{g}'] = np.ascontiguousarray(wih.T).astype(bf)
            pr[f'{pref}_b_{g}'] = bb
    # padded w^2 sets: 32 rows per perspective; tile a = w3..w6, tile b = w7, w8
    wsq_pad = np.zeros((6 * 32, H), np.float32)
    for i in range(6):
        wsq_pad[i * 32:i * 32 + L] = np.asarray(inputs[f'mp_w{i + 3}'], np.float32) ** 2
    pr['wsqT_a_f32'] = np.ascontiguousarray(wsq_pad[0:128].T)
    pr['wsqT_b_f32'] = np.ascontiguousarray(wsq_pad[128:192].T)
    pr['wsqT_a_bf16'] = pr['wsqT_a_f32'].astype(bf)
    pr['wsqT_b_bf16'] = pr['wsqT_b_f32'].astype(bf)
    return pr


def kernel(**inputs):
    if 'l1' not in _cache:
        _cache['l1'] = build_launch1()
        _cache['l2'] = build_launch2()
    nc1, nc2 = _cache['l1'], _cache['l2']

    pr = _prep_host(inputs)
    left = np.asarray(inputs['left'], np.float32)
    right = np.asarray(inputs['right'], np.float32)

    in_maps = []
    for b in range(B):
        for side in range(2):
            A = left[b] if side == 0 else right[b]
            Bx = right[b] if side == 0 else left[b]
            m = dict(pr)
            m['AT'] = np.ascontiguousarray(A.T)
            m['BT'] = np.ascontiguousarray(Bx.T)
            in_maps.append(m)

    res1 = run_bass_kernel_spmd(nc1, in_maps, list(range(8)), trace=TRACE)

    # assemble x [4, 1626]
    xs = []
    for b in range(B):
        rp = res1.results[2 * b]
        rh = res1.results[2 * b + 1]
        # agg_out holds 2h (tanh-trick encoding) — undo here
        ap_f = 0.5 * rp['agg_out'][:, 0, :].T.reshape(-1)
        ap_b = 0.5 * rp['agg_out'][:, 1, :].T.reshape(-1)
        ah_f = 0.5 * rh['agg_out'][:, 0, :].T.reshape(-1)
        ah_b = 0.5 * rh['agg_out'][:, 1, :].T.reshape(-1)
        meanL = rp['meanA']
        meanR = rh['meanA']
        xs.append(np.concatenate([ap_f, ap_b, ah_f, ah_b, [0.5, 0.5], meanL, meanR]))
    x = np.stack(xs).astype(np.float32)

    NX = 4 * H + 2 + 2 * D  # 1626
    NXP = NT2 * 128
    xTp = np.zeros((NXP, B), np.float32)
    xTp[0:NX] = x.T
    w1Tp = np.zeros((NXP, 2 * H), np.float32)
    w1Tp[0:NX] = np.asarray(inputs['fc1_W'], np.float32).T
    xbf = xTp.astype(ml_dtypes.bfloat16)
    w1bf = w1Tp.astype(ml_dtypes.bfloat16)
    b1f = np.asarray(inputs['fc1_b'], np.float32)
    w2Tf = np.ascontiguousarray(np.asarray(inputs['fc2_W'], np.float32).T)
    in_maps2 = []
    for ci in range(8):
        sl = slice(ci * NH2, (ci + 1) * NH2)
        in_maps2.append({
            'xT': xbf,
            'w1T': np.ascontiguousarray(w1bf[:, sl]),
            'b1': b1f[sl],
            'w2T': np.ascontiguousarray(w2Tf[sl]),
        })
    res2 = run_bass_kernel_spmd(nc2, in_maps2, list(range(8)))
    y = sum(r['yT'] for r in res2.results).T + np.asarray(inputs['fc2_b'], np.float32)
    _cache['last_exec_ns'] = (res1.exec_time_ns, res2.exec_time_ns)
    return np.ascontiguousarray(y.astype(np.float32))



# revision 41
# speedup vs baseline: 1.0277x; 1.0277x over previous
"""BiMPM forward on 8 Trainium2 NeuronCores (Bass/Tile).

Sharding: 8 cores = (batch b in 0..3) x (side in {p, h}).
  core 2b+0: A = left[b],  B = right[b]   -> mv_p features + agg over mv_p
  core 2b+1: A = right[b], B = left[b]    -> mv_h features + agg over mv_h
Every core runs the same program (SPMD) on its own (A, B) pair:
  ctx BiLSTM over A and B (fw group + bw group, 2 seqs batched per group),
  matching (62 A-side features, feature-major), agg BiLSTM over mv_A
  (final hidden states only). A tiny second launch computes the final FC
  from the gathered per-core agg states.

LSTM recurrence is weight-stationary: per step, 16 (LDWEIGHTS+MATMUL) pairs
produce g.T chunks [128, M] in one PSUM bank; gates evaluated in transposed
layout so h.T feeds the next step's matmul directly (no per-step transpose).
"""
import sys

sys.path.insert(0, '/opt/trn_rl_repo')

import numpy as np
import ml_dtypes

import concourse.bass as bass
import concourse.mybir as mybir
from concourse import tile, masks
from concourse.bass_utils import run_bass_kernel_spmd

F32 = mybir.dt.float32
BF16 = mybir.dt.bfloat16
FP8 = mybir.dt.float8e3  # e3m4
WSCALE = 32.0  # fp8 ctx Whh scale; folded into Wih/bias, undone by act scale
AF = mybir.ActivationFunctionType
OP = mybir.AluOpType
AX = mybir.AxisListType

EPS = 1e-8
B, S, D, H, L, NCLS = 4, 256, 300, 256, 10, 22
GH = 4 * H  # 1024 gates
NCHUNK = 8  # 1024 / 128
AGG_IN = 62
NEG_BIG = -3.0e38

DEBUG_OUTS = False
TRACE = False

# gate chunk order in PSUM columns: i0 i1 f0 f1 o0 o1 g0 g1 (sigmoid 0:6, tanh 6:8)
# host permutes weight/bias gate blocks accordingly (torch i f g o -> i f o g).


class PatchedTC(tile.TileContext):
    """This walrus build rejects instructions carrying more than MAX_WAITS sync
    waits. Tile freely attaches many (one per outstanding producer proc).
    After scheduling, split the excess onto same-engine NOP carriers placed
    immediately before the overloaded instruction."""


MAX_WAITS = 1


def _split_waits(nc, maxw=None):
    if maxw is None:
        maxw = MAX_WAITS
    for f in nc.m.functions:
        for blk in f.blocks:
            insts = blk.instructions  # live list
            out = []
            for inst in insts:
                si = getattr(inst, 'sync_info', None)
                waits = list(si.on_wait) if si is not None else []
                if len(waits) > maxw:
                    excess = waits[:-maxw]
                    for w0 in range(0, len(excess), maxw):
                        nop = _make_nop(nc, inst.engine)
                        nop.sync_info = mybir.SyncInfo(
                            on_wait=excess[w0:w0 + maxw], on_update=[])
                        out.append(nop)
                    inst.sync_info = mybir.SyncInfo(
                        on_wait=waits[-maxw:], on_update=list(si.on_update))
                out.append(inst)
            if len(out) != len(insts):
                insts.clear()
                insts.extend(out)


def _make_nop(nc, engine):
    bi = nc.engines[engine].nop(nofuse=True)
    inst = bi.ins
    cur = nc.cur_bb.bb.instructions
    assert cur and cur[-1].name == inst.name
    cur.pop()
    return inst


# ----------------------------------------------------------------------------
# launch 1 program
# ----------------------------------------------------------------------------

def build_launch1():
    nc = bass.Bass()

    dr = {}
    dr['AT'] = nc.dram_tensor('AT', [D, S], F32, kind='ExternalInput')
    dr['BT'] = nc.dram_tensor('BT', [D, S], F32, kind='ExternalInput')
    for g in range(2):  # 0=fw 1=bw
        dr[f'ctx_WihT_{g}'] = nc.dram_tensor(f'ctx_WihT_{g}', [D, GH], BF16, kind='ExternalInput')
        dr[f'ctx_WhhT_{g}'] = nc.dram_tensor(f'ctx_WhhT_{g}', [H, GH], FP8, kind='ExternalInput')
        dr[f'ctx_b_{g}'] = nc.dram_tensor(f'ctx_b_{g}', [GH], F32, kind='ExternalInput')
        dr[f'agg_WihT_{g}'] = nc.dram_tensor(f'agg_WihT_{g}', [AGG_IN, GH], BF16, kind='ExternalInput')
        dr[f'agg_WhhT_{g}'] = nc.dram_tensor(f'agg_WhhT_{g}', [H, GH], BF16, kind='ExternalInput')
        dr[f'agg_b_{g}'] = nc.dram_tensor(f'agg_b_{g}', [GH], F32, kind='ExternalInput')
    # w*w, padded to 32 cols per perspective: tile a = [w3 w4 w5 w6], tile b = [w7 w8]
    dr['wsqT_a_f32'] = nc.dram_tensor('wsqT_a_f32', [H, 128], F32, kind='ExternalInput')
    dr['wsqT_b_f32'] = nc.dram_tensor('wsqT_b_f32', [H, 64], F32, kind='ExternalInput')
    dr['wsqT_a_bf16'] = nc.dram_tensor('wsqT_a_bf16', [H, 128], BF16, kind='ExternalInput')
    dr['wsqT_b_bf16'] = nc.dram_tensor('wsqT_b_bf16', [H, 64], BF16, kind='ExternalInput')

    encB_dram = [nc.dram_tensor(f'encB_dram_{g}', [S, H], BF16) for g in range(2)]
    # rows staged for partition-broadcast: [2, S]: 0 rnB_cos, 1 rsumA_recip
    brow_dram = [nc.dram_tensor(f'brow_dram_{g}', [2, S], F32) for g in range(2)]
    # maxpool per-perspective B-side recip norms, staged for bulk broadcast
    rlrow_dram = [nc.dram_tensor(f'rlrow_dram_{g}', [L, S], F32) for g in range(2)]

    dr['agg_out'] = nc.dram_tensor('agg_out', [128, 2, 2], F32, kind='ExternalOutput')
    dr['meanA'] = nc.dram_tensor('meanA', [D], F32, kind='ExternalOutput')
    if DEBUG_OUTS:
        dr['mvT_dbg'] = nc.dram_tensor('mvT_dbg', [AGG_IN, S], F32, kind='ExternalOutput')
        dr['encA_dbg'] = nc.dram_tensor('encA_dbg', [2, 128, 2, S + 1], BF16, kind='ExternalOutput')
        dr['encB_dbg'] = nc.dram_tensor('encB_dbg', [2, 128, 2, S + 1], BF16, kind='ExternalOutput')

    with PatchedTC(nc) as tc:
        _emit_core_program(nc, tc, dr, encB_dram, brow_dram, rlrow_dram)
    _split_waits(nc)
    return nc


def _emit_core_program(nc, tc, dr, encB_dram, brow_dram, rlrow_dram):
    with tc.tile_pool(name='persist', bufs=1) as persist:
        # ---------------- identities, weights, inputs ----------------
        id_bf16 = persist.tile([128, 128], BF16, tag='idb', name='idb')
        id_f32 = persist.tile([128, 128], F32, tag='idf', name='idf')
        masks.make_identity(nc, id_bf16[:])
        masks.make_identity(nc, id_f32[:])

        kctx = [(0, 128), (128, 128), (256, 44)]
        # inputs + ctx wih first: the ~30 weight DMAs serialize on one queue
        # at ~650ns each, and preact only needs x + wih.
        xT, xTb = {}, {}
        for nm in ('A', 'B'):
            xT[nm], xTb[nm] = [], []
            for (k0, kn) in kctx:
                t = persist.tile([kn, S], F32, tag=f'x{nm}_{k0}', name=f'x{nm}_{k0}')
                nc.sync.dma_start(t[:], dr[f'{nm}T'][k0:k0 + kn, :])
                xT[nm].append(t)
                tb = persist.tile([kn, S], BF16, tag=f'xb{nm}_{k0}', name=f'xb{nm}_{k0}')
                nc.vector.tensor_copy(tb[:], t[:])
                xTb[nm].append(tb)

        wih, whh, bias = {}, {}, {}
        for g in range(2):
            wih[g] = []
            for (k0, kn) in kctx:
                t = persist.tile([kn, GH], BF16, tag=f'wih{g}_{k0}', name=f'wih{g}_{k0}')
                nc.sync.dma_start(t[:], dr[f'ctx_WihT_{g}'][k0:k0 + kn, :])
                wih[g].append(t)
            t = persist.tile([128, NCHUNK], F32, tag=f'bias{g}', name=f'bias{g}')
            nc.sync.dma_start(t[:], dr[f'ctx_b_{g}'].rearrange('(c p) -> p c', p=128))
            bias[g] = t
        for g in range(2):
            whh[g] = []
            for k in range(2):
                t = persist.tile([128, GH], FP8, tag=f'whh{g}_{k}', name=f'whh{g}_{k}')
                nc.sync.dma_start(t[:], dr[f'ctx_WhhT_{g}'][k * 128:(k + 1) * 128, :])
                whh[g].append(t)

        awih, awhh, abias = {}, {}, {}
        for g in range(2):
            t = persist.tile([AGG_IN, GH], BF16, tag=f'awih{g}', name=f'awih{g}')
            nc.sync.dma_start(t[:], dr[f'agg_WihT_{g}'][:])
            awih[g] = t
            awhh[g] = []
            for k in range(2):
                t = persist.tile([128, GH], BF16, tag=f'awhh{g}_{k}', name=f'awhh{g}_{k}')
                nc.sync.dma_start(t[:], dr[f'agg_WhhT_{g}'][k * 128:(k + 1) * 128, :])
                awhh[g].append(t)
            t = persist.tile([128, NCHUNK], F32, tag=f'abias{g}', name=f'abias{g}')
            nc.sync.dma_start(t[:], dr[f'agg_b_{g}'].rearrange('(c p) -> p c', p=128))
            abias[g] = t

        # wsq_f[ab][k], wsq_b[ab][k]: fp32/bf16 w^2 tiles; ab=0 -> 128 cols, ab=1 -> 64
        wsq_f, wsq_b = {}, {}
        for ab, nch in ((0, 128), (1, 64)):
            wsq_f[ab], wsq_b[ab] = [], []
            abn = 'a' if ab == 0 else 'b'
            for k in range(2):
                t = persist.tile([128, nch], F32, tag=f'wsqf{abn}{k}', name=f'wsqf{abn}{k}')
                nc.sync.dma_start(t[:], dr[f'wsqT_{abn}_f32'][k * 128:(k + 1) * 128, :])
                wsq_f[ab].append(t)
                t = persist.tile([128, nch], BF16, tag=f'wsqb{abn}{k}', name=f'wsqb{abn}{k}')
                nc.sync.dma_start(t[:], dr[f'wsqT_{abn}_bf16'][k * 128:(k + 1) * 128, :])
                wsq_b[ab].append(t)

        ones_col = persist.tile([128, 1], F32, tag='ones', name='ones')
        nc.vector.memset(ones_col[:], 1.0)
        ones_row = persist.tile([1, 128], F32, tag='onesr', name='onesr')
        nc.vector.memset(ones_row[:], 1.0)

        macc = persist.tile([128, 3], F32, tag='macc', name='macc')
        msc = persist.tile([128, 3], F32, tag='msc', name='msc')
        nc.vector.memset(macc[:], 0.0)
        for ki, (k0, kn) in enumerate(kctx):
            nc.vector.tensor_reduce(macc[0:kn, ki:ki + 1], xT['A'][ki][:], axis=AX.X, op=OP.add)
        nc.scalar.activation(msc[:], macc[:], AF.Copy, scale=1.0 / S)
        for ki, (k0, kn) in enumerate(kctx):
            nc.sync.dma_start(dr['meanA'][k0:k0 + kn], msc[0:kn, ki:ki + 1])

        # ---------------- ctx pre-activation ----------------
        # padded [*, WUP:WUP+S, *] live; WUP zero cols either side feed the
        # chunked scan's boundary-chain warmups with exact zero state.
        SP = S + 2 * WUP
        preT = {g: persist.tile([128, NCHUNK, SP, 2], F32, tag=f'pre{g}', name=f'pre{g}') for g in range(2)}
        with tc.tile_pool(name='prepsum', bufs=3, space='PSUM') as pp:
            for g in range(2):
                nc.vector.memset(preT[g][:, :, 0:WUP, :], 0.0)
                nc.vector.memset(preT[g][:, :, S + WUP:SP, :], 0.0)
                for c in range(NCHUNK):
                    ps = pp.tile([128, 2, S], F32, tag='preps', name='preps')
                    n_mm = 0
                    for s, nm in enumerate(('A', 'B')):
                        for ki in range(3):
                            nc.tensor.matmul(
                                ps[:, s, :], wih[g][ki][:, c * 128:(c + 1) * 128], xTb[nm][ki][:],
                                start=(n_mm == 0), stop=(n_mm == 5), skip_group_check=True)
                            n_mm += 1
                    for s in range(2):
                        nc.scalar.activation(
                            preT[g][:, c, WUP:WUP + S, s],
                            ps[:, s, :], AF.Identity, bias=bias[g][:, c:c + 1])

        # ---------------- ctx scans ----------------
        # encT[g]: [128, (half*seq 2M flat, col S+1)] bf16 — flat (hf, m) dim
        # so one strided stt writes all chains x halves x seqs per slot;
        # fw: h_t at col t+1, bw: h_t at col t. enc8: fp8 recurrence copy.
        encT = {g: persist.tile([128, 4, S + 1], BF16, tag=f'enc{g}', name=f'enc{g}') for g in range(2)}
        enc8 = {g: persist.tile([128, 4, S + 1], FP8, tag=f'enc8_{g}', name=f'enc8_{g}') for g in range(2)}

        _emit_scan_chunked(nc, tc, 'ctx', whh, preT, encT, M=2, final_out=None, id_f32=id_f32,
                           enc8=enc8, act_scale=1.0 / WSCALE, KCH=KCH_CTX)

        # ---------------- matching ----------------
        mvT = persist.tile([128, S], F32, tag='mvT', name='mvT')
        _emit_matching(nc, tc, dr, encT, encB_dram, brow_dram, rlrow_dram,
                       wsq_f, wsq_b, ones_col, ones_row, id_bf16, id_f32, mvT)
        mvTb = persist.tile([AGG_IN, S], BF16, tag='mvTb', name='mvTb')
        nc.vector.tensor_copy(mvTb[:], mvT[0:AGG_IN, :])

        if DEBUG_OUTS:
            nc.sync.dma_start(dr['mvT_dbg'][:], mvT[0:AGG_IN, :])
            for g in range(2):
                nc.sync.dma_start(dr['encA_dbg'][g], encT[g][:, 0])
                nc.sync.dma_start(dr['encB_dbg'][g], encT[g][:, 1])

        # ---------------- agg ----------------
        apreT = {g: persist.tile([128, NCHUNK, SP, 1], F32, tag=f'apre{g}', name=f'apre{g}') for g in range(2)}
        with tc.tile_pool(name='aggpp', bufs=3, space='PSUM') as pp:
            for g in range(2):
                nc.vector.memset(apreT[g][:, :, 0:WUP, :], 0.0)
                nc.vector.memset(apreT[g][:, :, S + WUP:SP, :], 0.0)
                for c in range(NCHUNK):
                    ps = pp.tile([128, S], F32, tag='apreps', name='apreps')
                    nc.tensor.matmul(ps[:], awih[g][:, c * 128:(c + 1) * 128], mvTb[:],
                                     start=True, stop=True)
                    # alternate evacuation between Act and DVE (16 copies total)
                    if c % 2 == 0:
                        nc.scalar.activation(apreT[g][:, c, WUP:WUP + S, 0], ps[:], AF.Identity,
                                             bias=abias[g][:, c:c + 1])
                    else:
                        nc.vector.tensor_scalar_add(apreT[g][:, c, WUP:WUP + S, 0], ps[:],
                                                    abias[g][:, c:c + 1])

        aencT = {g: persist.tile([128, 2, S + 1], BF16, tag=f'aenc{g}', name=f'aenc{g}') for g in range(2)}

        final_h = persist.tile([128, 2, 2], F32, tag='finalh', name='finalh')  # (group, half)
        _emit_scan_chunked(nc, tc, 'agg', awhh, apreT, aencT, M=1, final_out=final_h, id_f32=id_f32, KCH=KCH_AGG)
        nc.sync.dma_start(dr['agg_out'][:], final_h[:])


KCH = 16  # chunks per direction (must divide S)
KCH_CTX = 16
KCH_AGG = 16  # agg chunking is error-sensitive: 32 hit rel err 1.99e-2 on device
WUP = 16  # warmup steps per chunk; chunk-boundary state error decays ~0.67^WUP


def _emit_scan_chunked(nc, tc, name, whh, preT, encT, M, final_out, id_f32,
                       enc8=None, act_scale=1.0, KCH=KCH):
    """Lockstep chunked scan. Per direction, KCH independent chains (one per
    seq chunk) advance together, so each per-step engine op covers all KCH
    chains in one strided-AP instruction and the ~1.8us/step cross-engine
    dependence chain is amortized KCH-fold. Each chain runs WUP zero-state
    warmup steps on the real pre-activations preceding its chunk; chain 0
    fw (and the mirrored bw chain) warms up on preT's zero padding, which
    reproduces the exact zero initial state (zero pre -> c,h stay 0).

    preT[g]: [128, NCHUNK, S + 2*WUP, M], cols [0,WUP) and [S+WUP,S+2WUP)
    zero. encT/enc8 layouts unchanged; fw chain j owns real cols
    jC+1..jC+C (bw mirrored). Warmup h lives in a scratch block
    [128, M, 2, KCH, WUP+1] (col 0 = zero init), fp8 for ctx, bf16 agg.
    Gate math is the same tanh-trick as _emit_scan, KCH-wide."""
    C = S // KCH
    SLOTS = C + WUP
    with (
        tc.tile_pool(name=f'{name}_ps0', bufs=2, space='PSUM') as pp0,
        tc.tile_pool(name=f'{name}_ps1', bufs=2, space='PSUM') as pp1,
        tc.tile_pool(name=f'{name}_sb', bufs=3) as sb,
        tc.tile_pool(name=f'{name}_scr', bufs=1) as scrp,
    ):
        pps = {0: pp0, 1: pp1}
        rec_dt = FP8 if enc8 is not None else BF16
        rec = {g: (enc8[g] if enc8 is not None else encT[g]) for g in range(2)}
        # stt/activation outputs must be <=3D (birverifier): gate tiles are
        # [128, KCH, gate-cols] and enc writes are split per h-half.
        scr, c_state = {}, {}
        for g in range(2):
            t = scrp.tile([128, WUP + 1, KCH, 2 * M], rec_dt, tag=f'scr{g}', name=f'scr{g}')
            nc.vector.memset(t[:, 0, :, :], 0.0)
            scr[g] = t
            c_state[g] = scrp.tile([128, KCH, 2 * M], F32, tag=f'c{g}', name=f'c{g}')
            nc.vector.memset(c_state[g][:], 0.0)
        for s in range(SLOTS):
            for g in range(2):
                if g == 0:
                    pre_off = s
                    wr_off = s - WUP + 1
                    rd_off = s - WUP
                else:
                    pre_off = C - 1 + 2 * WUP - s
                    wr_off = C - 1 + WUP - s
                    rd_off = C + WUP - s
                nsl = (KCH - 1) * C + 1  # strided-slice span over chains
                ps = pps[g].tile([128, KCH, NCHUNK * M], F32, tag=f'gsum{g}', name=f'gsum{g}')
                for c in range(NCHUNK):
                    nc.tensor.matmul(
                        ps[:, :, c * M:(c + 1) * M], id_f32[:],
                        preT[g][:, c, pre_off:pre_off + nsl:C, :],
                        start=(c == 0), stop=False, skip_group_check=True)
                n_mm = 0
                for c in range(NCHUNK):
                    for k in range(2):
                        if s <= WUP:
                            rsrc = scr[g][:, s, :, k * M:(k + 1) * M]
                        else:
                            rsrc = rec[g][:, k * M:(k + 1) * M,
                                          rd_off:rd_off + nsl:C].transpose([0, 2, 1])
                        nc.tensor.matmul(
                            ps[:, :, c * M:(c + 1) * M],
                            whh[g][k][:, c * 128:(c + 1) * 128],
                            rsrc, start=False, stop=(n_mm == 15), skip_group_check=True)
                        n_mm += 1
                T = sb.tile([128, KCH, NCHUNK * M], F32, tag=f'th8{g}', name=f'th8{g}')
                nc.scalar.activation(T[:], ps[:], AF.Tanh, scale=act_scale)
                cs = c_state[g]
                Bt = sb.tile([128, KCH, 2 * M], F32, tag=f'B{g}', name=f'B{g}')
                nc.vector.scalar_tensor_tensor(
                    Bt[:], T[:, :, 0:2 * M], 1.0, T[:, :, 6 * M:8 * M],
                    op0=OP.add, op1=OP.mult)
                At = sb.tile([128, KCH, 2 * M], F32, tag=f'A{g}', name=f'A{g}')
                nc.vector.scalar_tensor_tensor(
                    At[:], T[:, :, 2 * M:4 * M], 1.0, cs[:], op0=OP.add, op1=OP.mult)
                nc.vector.scalar_tensor_tensor(
                    cs[:], At[:], 0.5, Bt[:], op0=OP.mult, op1=OP.add)
                TH = sb.tile([128, KCH, 2 * M], F32, tag=f'TH{g}', name=f'TH{g}')
                nc.scalar.activation(TH[:], cs[:], AF.Tanh, scale=0.5)
                if s < WUP:
                    # warmup: only the recurrence copy (fp8 ctx / bf16 agg)
                    nc.vector.scalar_tensor_tensor(
                        scr[g][:, s + 1, :, :], T[:, :, 4 * M:6 * M], 1.0, TH[:],
                        op0=OP.add, op1=OP.mult)
                else:
                    wsl = slice(wr_off, wr_off + nsl, C)
                    nc.vector.scalar_tensor_tensor(
                        rec[g][:, :, wsl].transpose([0, 2, 1]),
                        T[:, :, 4 * M:6 * M], 1.0, TH[:], op0=OP.add, op1=OP.mult)
                    if enc8 is not None:
                        nc.vector.scalar_tensor_tensor(
                            encT[g][:, :, wsl].transpose([0, 2, 1]),
                            T[:, :, 4 * M:6 * M], 1.0, TH[:], op0=OP.add, op1=OP.mult)
                if final_out is not None and s == SLOTS - 1:
                    ec = KCH - 1 if g == 0 else 0
                    nc.vector.scalar_tensor_tensor(
                        final_out[:, g, :], T[:, ec, 4 * M:6 * M], 1.0, TH[:, ec, :],
                        op0=OP.add, op1=OP.mult)


def _emit_scan(nc, tc, name, whh, preT, encT, M, final_out, id_f32,
               enc8=None, act_scale=1.0):
    """Interleaved fw/bw scan groups, tanh-only gate math.

    Host prep scales gate rows so that: i,f,o pre-acts arrive as x/2 and the
    whh matmul consumes the 2h encoding. Per step:
      PSUM prefilled with pre via identity matmuls, whh matmuls accumulate,
      T = tanh(psum)   (sig(x) = (T+1)/2 for i,f,o cols; tg for g cols)
      state W = 2c:  W' = 0.5*(t_f+1)*W + (t_i+1)*tg
      TH = tanh(0.5*W') = tanh(c')
      enc h2x = (t_o+1)*TH = 2h
    encT[g]: [128, (M seq, 2 half, S+1)] bf16 storing 2h. preT[g]: [128, (8, S, M)] f32.
    whh[g]: 2 k-tiles [128, 1024] bf16, gate chunks i0i1 f0f1 o0o1 g0g1,
    rows pre-scaled by 0.25 (i,f,o) / 0.5 (g)."""
    with (
        tc.tile_pool(name=f'{name}_ps0', bufs=2, space='PSUM') as pp0,
        tc.tile_pool(name=f'{name}_ps1', bufs=2, space='PSUM') as pp1,
        tc.tile_pool(name=f'{name}_sb', bufs=3) as sb,
    ):
        pps = {0: pp0, 1: pp1}
        c_state = {}
        for g in range(2):
            c_state[g] = sb.tile([128, 2 * M], F32, tag=f'c{g}', name=f'c{g}')  # (half, m) order
            nc.vector.memset(c_state[g][:], 0.0)
        for step in range(S):
            for g in range(2):
                t = step if g == 0 else S - 1 - step
                rd = t if g == 0 else t + 1
                wr = t + 1 if g == 0 else t
                ps = pps[g].tile([128, NCHUNK * M], F32, tag=f'gsum{g}', name=f'gsum{g}')
                psv = ps[:].rearrange('p (c m) -> p c m', c=NCHUNK)
                # prefill psum with pre (identity matmuls share one LDW)
                for c in range(NCHUNK):
                    nc.tensor.matmul(psv[:, c, :], id_f32[:], preT[g][:, c, t, :],
                                     start=(c == 0), stop=False, skip_group_check=True)
                rsrc = enc8[g] if enc8 is not None else encT[g]
                n_mm = 0
                for c in range(NCHUNK):
                    for k in range(2):
                        nc.tensor.matmul(
                            ps[:, c * M:(c + 1) * M],
                            whh[g][k][:, c * 128:(c + 1) * 128],
                            rsrc[:, :, k, rd],
                            start=False, stop=(n_mm == 15), skip_group_check=True)
                        n_mm += 1
                T = sb.tile([128, NCHUNK * M], F32, tag=f'th8{g}', name=f'th8{g}')
                nc.scalar.activation(T[:], ps[:], AF.Tanh, scale=act_scale)
                cs = c_state[g]
                A = sb.tile([128, 2 * M], F32, tag=f'A{g}', name=f'A{g}')
                nc.vector.scalar_tensor_tensor(
                    A[:], T[:, 2 * M:4 * M], 1.0, cs[:], op0=OP.add, op1=OP.mult)
                B = sb.tile([128, 2 * M], F32, tag=f'B{g}', name=f'B{g}')
                nc.vector.scalar_tensor_tensor(
                    B[:], T[:, 0:2 * M], 1.0, T[:, 6 * M:8 * M], op0=OP.add, op1=OP.mult)
                nc.vector.scalar_tensor_tensor(
                    cs[:], A[:], 0.5, B[:], op0=OP.mult, op1=OP.add)
                TH = sb.tile([128, 2 * M], F32, tag=f'TH{g}', name=f'TH{g}')
                nc.scalar.activation(TH[:], cs[:], AF.Tanh, scale=0.5)
                # h2x = (t_o+1)*TH; encT dest dims (m, h) permuted to (h, m)
                if enc8 is not None:
                    # fp8 copy feeds the next step's matmul — emit first (on
                    # the critical chain); bf16 copy (for matching) follows.
                    nc.vector.scalar_tensor_tensor(
                        enc8[g][:, :, :, wr].transpose([0, 2, 1]),
                        T[:, 4 * M:6 * M].rearrange('p (h m) -> p h m', h=2),
                        1.0, TH[:].rearrange('p (h m) -> p h m', h=2),
                        op0=OP.add, op1=OP.mult)
                hout = encT[g][:, :, :, wr].transpose([0, 2, 1])
                nc.vector.scalar_tensor_tensor(
                    hout, T[:, 4 * M:6 * M].rearrange('p (h m) -> p h m', h=2),
                    1.0, TH[:].rearrange('p (h m) -> p h m', h=2),
                    op0=OP.add, op1=OP.mult)
                if final_out is not None and step == S - 1:
                    # 2h; host scales agg_out by 0.5
                    nc.vector.scalar_tensor_tensor(
                        final_out[:, g, :], T[:, 4 * M:6 * M], 1.0, TH[:],
                        op0=OP.add, op1=OP.mult)


def _emit_matching(nc, tc, dr, encT, encB_dram, brow_dram, rlrow_dram,
                   wsq_f, wsq_b, ones_col, ones_row, id_bf16, id_f32, mvT):
    """A-side matching features into mvT rows 0:62 ([feat, S] f32).

    rows: 0 cos-max, 1 cos-mean, 2:12 maxpool-f(w3), 12:22 maxpool-b(w4),
          22:32 attentive-f(w5), 32:42 attentive-b(w6),
          42:52 max-attentive-f(w7), 52:62 max-attentive-b(w8)
    w-set s in 0..5 lives in wsq tile s//4 at col offset 32*(s%4), 10 cols wide.

    Emitted as phase A (norms, att, transposes, attentive-mean) for both
    directions, then phase B (maxpool, max-attentive, final cosines) for
    both: B's engine queues then never drain while the other direction's
    A-phase DMA chains are in flight.
    """
    with tc.tile_pool(name='m_sb', bufs=1) as msb:
        colfeat = msb.tile([128, 2, 22], F32, tag='colfeat', name='colfeat')
        att = {}
        rl_all = {}
        nsA = {}
        rnAs0 = {}
        meanT = {}

        for g in range(2):
            c0 = 1 if g == 0 else 0
            eAT = [encT[g][:, 2 * k, c0:c0 + S] for k in range(2)]  # [128, S] bf16 views
            eBT = [encT[g][:, 2 * k + 1, c0:c0 + S] for k in range(2)]

            # --- squares (scalar engine: DVE is the matching bottleneck)
            sqA = [msb.tile([128, S], F32, tag=f'sqA{k}', name=f'sqA{k}') for k in range(2)]
            sqB = [msb.tile([128, S], F32, tag=f'sqB{k}', name=f'sqB{k}') for k in range(2)]
            for k in range(2):
                nc.scalar.activation(sqA[k][:], eAT[k], AF.Square)
                nc.scalar.activation(sqB[k][:], eBT[k], AF.Square)

            # --- norm sets: nsqA/B for all 6 w-sets (padded 32 rows each, 2 tiles)
            # recip norms only for set tile 0 (maxpool); sets 2..5 use nsA raw.
            nsA[g] = [msb.tile([128, S], F32, tag=f'nsA{g}{ab}', name=f'nsA{g}{ab}') for ab in range(2)]
            rnAs0[g] = msb.tile([128, S], F32, tag=f'rnAs0{g}', name=f'rnAs0{g}')
            rnBs0 = msb.tile([128, S], F32, tag='rnBs0', name='rnBs0')
            with tc.tile_pool(name=f'mns{g}', bufs=2, space='PSUM') as mps:
                for ab, nch in ((0, 128), (1, 64)):
                    ps = mps.tile([128, S], F32, tag='nset', name='nset')
                    for k in range(2):
                        nc.tensor.matmul(ps[0:nch, :], wsq_f[ab][k][:], sqA[k][:],
                                         start=(k == 0), stop=(k == 1), skip_group_check=True)
                    nc.scalar.copy(nsA[g][ab][0:nch, :], ps[0:nch, :])
                    if ab == 0:
                        nc.scalar.activation(rnAs0[g][:], ps[:], AF.Sqrt)
                        nc.vector.tensor_scalar_max(rnAs0[g][:], rnAs0[g][:], EPS)
                        nc.vector.reciprocal(rnAs0[g][:], rnAs0[g][:])
                        ps2 = mps.tile([128, S], F32, tag='nset', name='nset')
                        for k in range(2):
                            nc.tensor.matmul(ps2[:], wsq_f[0][k][:], sqB[k][:],
                                             start=(k == 0), stop=(k == 1), skip_group_check=True)
                        nc.scalar.activation(rnBs0[:], ps2[:], AF.Sqrt)
                        nc.vector.tensor_scalar_max(rnBs0[:], rnBs0[:], EPS)
                        nc.vector.reciprocal(rnBs0[:], rnBs0[:])

            # --- stage + bulk-broadcast the maxpool B-side recip norms (set g):
            # one [L, S] DRAM write + two 5-row broadcasts (a 10-row f32 one
            # exceeds the ISA length limit), hoisted out of the maxpool loop.
            base = 32 * g
            rl_all[g] = msb.tile([128, L, S], F32, tag=f'rlall{g}', name=f'rlall{g}')
            nc.sync.dma_start(rlrow_dram[g][:], rnBs0[base:base + L, :])
            for hl in range(2):
                nc.sync.dma_start(
                    rl_all[g][:, hl * 5:(hl + 1) * 5, :],
                    rlrow_dram[g][hl * 5:(hl + 1) * 5, :].partition_broadcast(128))

            # --- cos recip norms: rnA [p,1] per-partition; rnB as a [1, S]
            # row (PE column-sum) then PE-broadcast to [128, S] PSUM — no
            # DRAM round trip (the old brow chain cost ~4us of DMA latency)
            rnA = msb.tile([128, 2], F32, tag='rnA', name='rnA')
            rnB_row = msb.tile([1, S], F32, tag='rnBrow', name='rnBrow')
            with tc.tile_pool(name=f'mn2{g}', bufs=2, space='PSUM') as mps:
                ps = mps.tile([128, 2], F32, tag='nsq', name='nsq')
                for pt in range(2):
                    for k in range(2):
                        nc.tensor.matmul(ps[:, pt:pt + 1],
                                         sqA[k][:, pt * 128:(pt + 1) * 128],
                                         ones_col[:], start=(k == 0), stop=(k == 1),
                                         skip_group_check=True)
                sq_ = msb.tile([128, 2], F32, tag='nsq_s', name='nsq_s')
                nc.scalar.activation(sq_[:], ps[:], AF.Sqrt)
                nc.vector.tensor_scalar_max(sq_[:], sq_[:], EPS)
                nc.vector.reciprocal(rnA[:], sq_[:])
                psr = mps.tile([1, S], F32, tag='nsqr', name='nsqr')
                for k in range(2):
                    nc.tensor.matmul(psr[:], ones_col[:], sqB[k][:],
                                     start=(k == 0), stop=(k == 1), skip_group_check=True)
                nc.scalar.activation(rnB_row[:], psr[:], AF.Sqrt)
                nc.vector.tensor_scalar_max(rnB_row[:], rnB_row[:], EPS)
                nc.vector.reciprocal(rnB_row[:], rnB_row[:])

            # --- att = num * rnA[p] * rnB[q]
            att[g] = [msb.tile([128, S], F32, tag=f'att{g}{pt}', name=f'att{g}{pt}')
                      for pt in range(2)]
            rsum = msb.tile([128, 2], F32, tag='rsum', name='rsum')
            with (
                tc.tile_pool(name=f'ma{g}', bufs=2, space='PSUM') as mps,
                tc.tile_pool(name=f'mab{g}', bufs=1, space='PSUM') as bcp,
            ):
                rnB_ps = bcp.tile([128, S], F32, tag='rnBps', name='rnBps')
                nc.tensor.matmul(rnB_ps[:], ones_row[:], rnB_row[:],
                                 start=True, stop=True, skip_group_check=True)
                # stt may read only one PSUM operand; evacuate the broadcast
                rnB_bc = msb.tile([128, S], F32, tag='rnBbc', name='rnBbc')
                nc.scalar.copy(rnB_bc[:], rnB_ps[:])
                for pt in range(2):
                    nps = mps.tile([128, S], F32, tag='num', name='num')
                    for k in range(2):
                        nc.tensor.matmul(nps[:], eAT[k][:, pt * 128:(pt + 1) * 128],
                                         eBT[k], start=(k == 0), stop=(k == 1),
                                         skip_group_check=True)
                    nc.vector.scalar_tensor_tensor(
                        att[g][pt][:], nps[:], rnA[:, pt:pt + 1], rnB_bc[:],
                        op0=OP.mult, op1=OP.mult)
                for pt in range(2):
                    if g == 0:  # cos max/mean features use att_fw only
                        nc.vector.tensor_reduce(colfeat[:, pt, 0:1], att[g][pt][:],
                                                axis=AX.X, op=OP.max)
                    nc.vector.tensor_reduce(rsum[:, pt:pt + 1], att[g][pt][:],
                                            axis=AX.X, op=OP.add)
                if g == 0:
                    nc.scalar.activation(colfeat[:, :, 1], rsum[:], AF.Copy, scale=1.0 / S)
                # NOTE: the attentive-mean 1/max(att.sum, EPS) factor is a
                # positive per-row scalar that cancels in the downstream
                # cosine (mp_match) — no need to compute or apply it.

            # --- transposes: enc_B [q, h] + attT [q, p]; both halves land in
            # one PSUM tile so each needs a single evacuation copy
            encB = [msb.tile([128, S], BF16, tag=f'encB{qt}', name=f'encB{qt}') for qt in range(2)]
            attT = [msb.tile([128, S], BF16, tag=f'attT{qt}', name=f'attT{qt}') for qt in range(2)]
            with tc.tile_pool(name=f'mt{g}', bufs=2, space='PSUM') as mps:
                for qt in range(2):
                    tpb = mps.tile([128, 2, 128], BF16, tag='tpb', name='tpb')
                    for hf in range(2):
                        nc.tensor.transpose(tpb[:, hf, :], eBT[hf][:, qt * 128:(qt + 1) * 128],
                                            id_bf16[:])
                    nc.scalar.copy(encB[qt][:], tpb[:].rearrange('p h n -> p (h n)'))
                    tpf = mps.tile([128, 2, 128], F32, tag='tpf', name='tpf')
                    for pt in range(2):
                        nc.tensor.transpose(tpf[:, pt, :], att[g][pt][:, qt * 128:(qt + 1) * 128],
                                            id_f32[:])
                    nc.scalar.copy(attT[qt][:], tpf[:].rearrange('p h n -> p (h n)'))
                    nc.sync.dma_start(encB_dram[g][qt * 128:(qt + 1) * 128, :], encB[qt][:])

            # --- attentive mean (transposed, unnormalized): meanT[h, p]
            meanT[g] = [msb.tile([128, S], BF16, tag=f'meanT{g}{ht}', name=f'meanT{g}{ht}')
                        for ht in range(2)]
            with tc.tile_pool(name=f'mm{g}', bufs=2, space='PSUM') as mps:
                for ht in range(2):
                    mp = mps.tile([128, S], F32, tag='meanps', name='meanps')
                    for qt in range(2):
                        nc.tensor.matmul(mp[:], encB[qt][:, ht * 128:(ht + 1) * 128], attT[qt][:],
                                         start=(qt == 0), stop=(qt == 1), skip_group_check=True)
                    nc.scalar.copy(meanT[g][ht][:], mp[:])

        # ---- phase B: one fused instruction stream per direction.
        # The max-attentive units keep Act/Pool/DVE all saturated; the
        # DVE-only maxpool iterations and the previous direction's final
        # cosine sets are sprinkled between vb blocks so the engines never
        # drain at section boundaries.
        QB = 16
        pend = []  # deferred emitters from the previous direction

        def _emit_xt_transpose(g, xacc_g, xT_g):
            with tc.tile_pool(name=f'mxt{g}', bufs=2, space='PSUM') as mps:
                for ht in range(2):
                    for pt in range(2):
                        tp = mps.tile([128, 128], BF16, tag='tpx', name='tpx')
                        nc.tensor.transpose(tp[:], xacc_g[pt][:, ht * 128:(ht + 1) * 128],
                                            id_bf16[:])
                        nc.scalar.copy(xT_g[ht][:, pt * 128:(pt + 1) * 128], tp[:])

        def _emit_final_set(g, eAT_g, vT, set_, row0):
            ab, off = divmod(set_, 4)
            off *= 32
            prod = [msb.tile([128, S], BF16, tag=f'prod{k}', name=f'prod{k}') for k in range(2)]
            vsq = [msb.tile([128, S], F32, tag=f'vsq{k}', name=f'vsq{k}') for k in range(2)]
            for k in range(2):
                nc.vector.tensor_tensor(prod[k][:], eAT_g[k], vT[k][:], OP.mult)
                nc.scalar.activation(vsq[k][:], vT[k][:], AF.Square)
            # stage this set's A-norms at base partition 0 (engine ops need equal bases)
            n1s = msb.tile([L, S], F32, tag='n1s', name='n1s')
            nc.sync.dma_start(n1s[:], nsA[g][ab][off:off + L, :])
            feat = msb.tile([L, S], F32, tag='feat', name='feat')
            with tc.tile_pool(name=f'mf{g}{row0}', bufs=1, space='PSUM') as mps:
                nump = mps.tile([128, S], F32, tag='nump', name='nump')
                n2p = mps.tile([128, S], F32, tag='n2p', name='n2p')
                for k in range(2):
                    nc.tensor.matmul(nump[0:L, :], wsq_b[ab][k][:, off:off + L],
                                     prod[k][:], start=(k == 0), stop=(k == 1),
                                     skip_group_check=True)
                    nc.tensor.matmul(n2p[0:L, :], wsq_f[ab][k][:, off:off + L],
                                     vsq[k][:], start=(k == 0), stop=(k == 1),
                                     skip_group_check=True)
                den = msb.tile([128, S], F32, tag='den', name='den')
                nc.vector.tensor_tensor(den[0:L, :], n2p[0:L, :], n1s[:], OP.mult)
                nc.scalar.activation(den[0:L, :], den[0:L, :], AF.Sqrt)
                nc.vector.tensor_scalar_max(den[0:L, :], den[0:L, :], EPS)
                nc.vector.reciprocal(den[0:L, :], den[0:L, :])
                nc.vector.tensor_tensor(feat[:], nump[0:L, :], den[0:L, :], OP.mult)
            # place rows via DMA (arbitrary partition offset)
            nc.sync.dma_start(mvT[row0:row0 + L, :], feat[:])

        for g in range(2):
            c0 = 1 if g == 0 else 0
            eAT = [encT[g][:, 2 * k, c0:c0 + S] for k in range(2)]
            eBT = [encT[g][:, 2 * k + 1, c0:c0 + S] for k in range(2)]
            base = 32 * g

            rnA_l = msb.tile([128, 2, L], F32, tag='rnAl', name='rnAl')
            mp_acc = msb.tile([128, 2, L], F32, tag='mpacc', name='mpacc')
            scr = msb.tile([128, S], F32, tag='mpscr', name='mpscr')
            xacc = [msb.tile([128, S], BF16, tag=f'xacc{g}{pt}', name=f'xacc{g}{pt}')
                    for pt in range(2)]
            acc2 = [msb.tile([128, 2, S], BF16, tag=f'acc2_{pt}', name=f'acc2_{pt}')
                    for pt in range(2)]
            for pt in range(2):
                nc.vector.memset(acc2[pt][:], NEG_BIG)
            # q-pairs share one [128, 2, H] tmp tile (Act/Pool/DVE each fill a
            # slot) and ONE DVE TT-max merges both units at 327ns/pair; per 16
            # pairs: 13 (A,P) + 2 (A,D) + 1 (D,D) -> Act 15x398, Pool 13x450,
            # DVE 16x327+4x127 -- all three within 4%.
            PAIRS = ['AP'] * 13 + ['AD'] * 2 + ['DD']  # 16 q-pairs / 32 units
            with (
                tc.tile_pool(name=f'mp{g}', bufs=3, space='PSUM') as mps,
                tc.tile_pool(name=f'mpb{g}', bufs=2) as bcp,
                tc.tile_pool(name=f'mx{g}', bufs=2) as vbp,
                tc.tile_pool(name=f'mxt{g}t', bufs=12) as tmpp,
            ):
                for pt in range(2):
                    tpf = mps.tile([128, L], F32, tag='tprn', name='tprn')
                    nc.tensor.transpose(tpf[:], rnAs0[g][base:base + L, pt * 128:(pt + 1) * 128],
                                        id_f32[base:base + L, base:base + L])
                    nc.scalar.copy(rnA_l[:, pt, :], tpf[:])
                for bi, q0 in enumerate(range(0, S, QB)):
                    vb = vbp.tile([128, QB, H], BF16, tag='vbc', name='vbc')
                    nc.sync.dma_start(vb[:], encB_dram[g][q0:q0 + QB, :].partition_broadcast(128))
                    for qp in range(QB // 2):
                        qq0, qq1 = 2 * qp, 2 * qp + 1
                        for pt in range(2):
                            kind = PAIRS[(2 * qp + pt) % 16]
                            tmp2 = tmpp.tile([128, 2, H], BF16, tag='xtmp', name='xtmp')
                            for sl, (qq, r) in enumerate(((qq0, kind[0]), (qq1, kind[1]))):
                                q = q0 + qq
                                if r == 'A':
                                    nc.scalar.activation(tmp2[:, sl, :], vb[:, qq, :], AF.Copy,
                                                         scale=att[g][pt][:, q:q + 1])
                                elif r == 'P':
                                    nc.gpsimd.tensor_scalar(tmp2[:, sl, :], vb[:, qq, :],
                                                            att[g][pt][:, q:q + 1], None,
                                                            op0=OP.mult)
                                else:
                                    nc.vector.tensor_scalar(tmp2[:, sl, :], vb[:, qq, :],
                                                            att[g][pt][:, q:q + 1], None,
                                                            op0=OP.mult)
                            nc.vector.tensor_tensor(acc2[pt][:], tmp2[:], acc2[pt][:], OP.max)
                    if 1 <= bi <= L:  # one maxpool perspective per vb block
                        l = bi - 1
                        wa = [bcp.tile([128, S], BF16, tag=f'wa{k}', name=f'wa{k}')
                              for k in range(2)]
                        for k in range(2):
                            nc.vector.tensor_scalar_mul(
                                wa[k][:], eAT[k], wsq_f[0][k][:, base + l:base + l + 1])
                        for pt in range(2):
                            nps = mps.tile([128, S], F32, tag='mpnum', name='mpnum')
                            for k in range(2):
                                nc.tensor.matmul(nps[:], wa[k][:, pt * 128:(pt + 1) * 128],
                                                 eBT[k], start=(k == 0), stop=(k == 1),
                                                 skip_group_check=True)
                            # (tensor_tensor_reduce would fuse these, but this
                            # walrus build rejects it: "ISA wrong length")
                            nc.vector.tensor_tensor(scr[:], nps[:], rl_all[g][:, l, :], OP.mult)
                            nc.vector.tensor_reduce(mp_acc[:, pt, l:l + 1], scr[:],
                                                    axis=AX.X, op=OP.max)
                    elif bi == L + 1:
                        for pt in range(2):
                            nc.vector.tensor_tensor(colfeat[:, pt, 2 + g * L:2 + (g + 1) * L],
                                                    mp_acc[:, pt, :], rnA_l[:, pt, :], OP.mult)
                    elif pend:
                        pend.pop(0)()
                for pt in range(2):
                    nc.vector.tensor_tensor(xacc[pt][:], acc2[pt][:, 0, :],
                                            acc2[pt][:, 1, :], OP.max)
            while pend:
                pend.pop(0)()

            # defer this direction's xacc transposes + final cosine sets into
            # the next stream (or the tail, for the last direction)
            xT_ = [msb.tile([128, S], BF16, tag=f'xT{ht}', name=f'xT{ht}') for ht in range(2)]
            pend = [
                (lambda g=g, xacc=xacc, xT_=xT_: _emit_xt_transpose(g, xacc, xT_)),
                (lambda g=g, eAT=eAT: _emit_final_set(g, eAT, meanT[g], 2 + g, 22 + g * L)),
                (lambda g=g, eAT=eAT, xT_=xT_: _emit_final_set(g, eAT, xT_, 4 + g, 42 + g * L)),
            ]
        while pend:
            pend.pop(0)()

        # --- transpose column features into mvT rows 0:22
        with tc.tile_pool(name='cf_ps', bufs=2, space='PSUM') as cps:
            for pt in range(2):
                tp = cps.tile([22, 128], F32, tag='tpcf', name='tpcf')
                nc.tensor.transpose(tp[:], colfeat[:, pt, :], id_f32[:])
                nc.scalar.copy(mvT[0:22, pt * 128:(pt + 1) * 128], tp[:])

NT2 = 13  # launch2 k-tiles; host pads NX 1626 -> 13*128
NH2 = 64  # hidden chunk per core: 512 / 8


def build_launch2():
    """Final FC, sharded over the 8 cores: core i computes its 64-wide slice
    of h = tanh(W1 x + b1) and the partial y = W2[:, slice] @ h_slice; the
    host sums the 8 partials (+b2). y is linear in h, so partials add."""
    nc = bass.Bass()
    NXP = NT2 * 128  # 1664 (zero-padded)
    xT = nc.dram_tensor('xT', [NXP, B], BF16, kind='ExternalInput')
    w1T = nc.dram_tensor('w1T', [NXP, NH2], BF16, kind='ExternalInput')
    b1 = nc.dram_tensor('b1', [NH2], F32, kind='ExternalInput')
    w2T = nc.dram_tensor('w2T', [NH2, NCLS], F32, kind='ExternalInput')
    yT = nc.dram_tensor('yT', [NCLS, B], F32, kind='ExternalOutput')

    with PatchedTC(nc) as tc:
        with (
            tc.tile_pool(name='sb', bufs=1) as sb,
            tc.tile_pool(name='ps', bufs=2, space='PSUM') as pp,
        ):
            xts = sb.tile([128, NT2, B], BF16, tag='x', name='x')
            nc.sync.dma_start(xts[:], xT.rearrange('(t p) n -> p t n', p=128))
            w1s = sb.tile([128, NT2, NH2], BF16, tag='w1', name='w1')
            nc.sync.dma_start(w1s[:], w1T.rearrange('(t p) n -> p t n', p=128))
            b1t = sb.tile([NH2, 1], F32, tag='b1', name='b1')
            nc.sync.dma_start(b1t[:], b1.rearrange('(n o) -> n o', o=1))
            w2s = sb.tile([NH2, NCLS], F32, tag='w2', name='w2')
            nc.sync.dma_start(w2s[:], w2T[:, :])
            hT = sb.tile([NH2, B], F32, tag='hT', name='hT')
            ps = pp.tile([NH2, B], F32, tag='h', name='h')
            for i in range(NT2):
                nc.tensor.matmul(ps[:], w1s[:, i, :], xts[:, i, :],
                                 start=(i == 0), stop=(i == NT2 - 1),
                                 skip_group_check=True)
            nc.scalar.activation(hT[:], ps[:], AF.Tanh, bias=b1t[:])
            ps2 = pp.tile([NCLS, B], F32, tag='y', name='y')
            nc.tensor.matmul(ps2[:], w2s[:], hT[:], start=True, stop=True,
                             skip_group_check=True)
            yt = sb.tile([NCLS, B], F32, tag='yt', name='yt')
            nc.scalar.copy(yt[:], ps2[:])
            nc.sync.dma_start(yT[:], yt[:])
    _split_waits(nc)
    return nc


# ----------------------------------------------------------------------------
# host orchestration
# ----------------------------------------------------------------------------

_cache = {}


def _gate_perm():
    # torch gate order (i, f, g, o) blocks of H -> chip order (i, f, o, g),
    # and within each gate the two 128-halves stay in order.
    idx = np.arange(GH).reshape(4, H)
    return np.concatenate([idx[0], idx[1], idx[3], idx[2]])


def _prep_host(inputs):
    bf = ml_dtypes.bfloat16
    perm = _gate_perm()
    # tanh-trick row scaling in chip gate order (i, f, o, g blocks of H):
    # Wih/bias rows: x/2 for sigmoid gates (i,f,o); Whh additionally consumes
    # the 2h encoding, so its rows get an extra 0.5.
    sig_rows = np.concatenate([np.full(3 * H, 0.5, np.float32),
                               np.ones(H, np.float32)])
    pr = {}
    for g, d in ((0, 'f'), (1, 'b')):
        for pref in ('ctx', 'agg'):
            wih = np.asarray(inputs[f'{pref}_Wih_{d}'], np.float32)[perm]  # [1024, IN]
            whh = np.asarray(inputs[f'{pref}_Whh_{d}'], np.float32)[perm]
            bb = np.asarray(inputs[f'{pref}_b_{d}'], np.float32)[perm]
            wih = wih * sig_rows[:, None]
            whh = whh * (0.5 * sig_rows)[:, None]
            bb = bb * sig_rows
            if pref == 'ctx':
                # fp8 recurrence: whh scaled up by WSCALE (kept in fp8 normal
                # range); wih/bias scaled to match, undone by the act scale.
                wih = wih * WSCALE
                bb = bb * WSCALE
                pr[f'{pref}_WhhT_{g}'] = np.ascontiguousarray(
                    (whh * WSCALE).T).astype(ml_dtypes.float8_e3m4)
            else:
                pr[f'{pref}_WhhT_{g}'] = np.ascontiguousarray(whh.T).astype(bf)
            pr[f'{pref}_WihT_{g}'] = np.ascontiguousarray(wih.T).astype(bf)
            pr[f'{pref}_b_{g}'] = bb
    # padded w^2 sets: 32 rows per perspective; tile a = w3..w6, tile b = w7, w8
    wsq_pad = np.zeros((6 * 32, H), np.float32)
    for i in range(6):
        wsq_pad[i * 32:i * 32 + L] = np.asarray(inputs[f'mp_w{i + 3}'], np.float32) ** 2
    pr['wsqT_a_f32'] = np.ascontiguousarray(wsq_pad[0:128].T)
    pr['wsqT_b_f32'] = np.ascontiguousarray(wsq_pad[128:192].T)
    pr['wsqT_a_bf16'] = pr['wsqT_a_f32'].astype(bf)
    pr['wsqT_b_bf16'] = pr['wsqT_b_f32'].astype(bf)
    return pr


def kernel(**inputs):
    if 'l1' not in _cache:
        _cache['l1'] = build_launch1()
        _cache['l2'] = build_launch2()
    nc1, nc2 = _cache['l1'], _cache['l2']

    pr = _prep_host(inputs)
    left = np.asarray(inputs['left'], np.float32)
    right = np.asarray(inputs['right'], np.float32)

    in_maps = []
    for b in range(B):
        for side in range(2):
            A = left[b] if side == 0 else right[b]
            Bx = right[b] if side == 0 else left[b]
            m = dict(pr)
            m['AT'] = np.ascontiguousarray(A.T)
            m['BT'] = np.ascontiguousarray(Bx.T)
            in_maps.append(m)

    res1 = run_bass_kernel_spmd(nc1, in_maps, list(range(8)), trace=TRACE)

    # assemble x [4, 1626]
    xs = []
    for b in range(B):
        rp = res1.results[2 * b]
        rh = res1.results[2 * b + 1]
        # agg_out holds 2h (tanh-trick encoding) — undo here
        ap_f = 0.5 * rp['agg_out'][:, 0, :].T.reshape(-1)
        ap_b = 0.5 * rp['agg_out'][:, 1, :].T.reshape(-1)
        ah_f = 0.5 * rh['agg_out'][:, 0, :].T.reshape(-1)
        ah_b = 0.5 * rh['agg_out'][:, 1, :].T.reshape(-1)
        meanL = rp['meanA']
        meanR = rh['meanA']
        xs.append(np.concatenate([ap_f, ap_b, ah_f, ah_b, [0.5, 0.5], meanL, meanR]))
    x = np.stack(xs).astype(np.float32)

    NX = 4 * H + 2 + 2 * D  # 1626
    NXP = NT2 * 128
    xTp = np.zeros((NXP, B), np.float32)
    xTp[0:NX] = x.T
    w1Tp = np.zeros((NXP, 2 * H), np.float32)
    w1Tp[0:NX] = np.asarray(inputs['fc1_W'], np.float32).T
    xbf = xTp.astype(ml_dtypes.bfloat16)
    w1bf = w1Tp.astype(ml_dtypes.bfloat16)
    b1f = np.asarray(inputs['fc1_b'], np.float32)
    w2Tf = np.ascontiguousarray(np.asarray(inputs['fc2_W'], np.float32).T)
    in_maps2 = []
    for ci in range(8):
        sl = slice(ci * NH2, (ci + 1) * NH2)
        in_maps2.append({
            'xT': xbf,
            'w1T': np.ascontiguousarray(w1bf[:, sl]),
            'b1': b1f[sl],
            'w2T': np.ascontiguousarray(w2Tf[sl]),
        })
    res2 = run_bass_kernel_spmd(nc2, in_maps2, list(range(8)))
    y = sum(r['yT'] for r in res2.results).T + np.asarray(inputs['fc2_b'], np.float32)
    _cache['last_exec_ns'] = (res1.exec_time_ns, res2.exec_time_ns)
    return np.ascontiguousarray(y.astype(np.float32))

